# revision 38
# baseline (speedup 1.0000x reference)
"""GAT (2-layer, 6-head) + GraphNorm + readout MLP on 8 Trainium2 cores.

Sharding: graph-level data parallelism. 48 fixed-size graphs (228 nodes,
edges never cross graphs) -> 6 graphs per core. Weights replicated.

v2 redesign vs the per-graph baseline:
  - All 6 graphs batched per stage; channel-major [c, (g, n)] primary layout.
  - Dense attention scores z[s,d] built with GpSimd partition-broadcast of
    the a2 rows + one wide DVE add per graph ([114, 2*6*228] bf16 tiles,
    both source-halves per instruction), lrelu on DVE, exp on Scalar
    (single activation table: exp/ln/relu/copy), multiplicity mask on GpSimd.
  - Attention logits a1/a2 computed straight from the layer input with
    host-folded was = W @ [as|ad].
  - Aggregation: dest-partition matmuls with a fused ones-column so the
    softmax denominator falls out of the same PSUM tile.
  - lin1 readout: weights host-reordered to (ck, n, p) chunks of k=128 so
    the GEMV consumes the channel-major layer-2 output directly; weights
    streamed in 4 double-buffered DMA chunks overlapping the layer phase.

kernel(**inputs) -> np.ndarray [48, 2] float32.
"""
import sys
sys.path.insert(0, '/opt/trn_rl_repo')

import numpy as np

import concourse.bass as bass
import concourse.bacc as bacc
import concourse.mybir as mybir
import concourse.tile as tile
from concourse import masks
from concourse import bass_utils

F32 = mybir.dt.float32
BF16 = mybir.dt.bfloat16
Alu = mybir.AluOpType
Act = mybir.ActivationFunctionType

H, C = 6, 64
HC = 384
NPG = 228          # nodes per graph
B = 48             # graphs
GPC = 6            # graphs per core
NCORES = 8
F_IN = 228
NH = 114           # node half-chunk
NCLS = 2
NG = GPC * NPG     # 1368 node-columns per core
NJ1 = 3 * NPG      # 684 lin1 k-chunks of 128
NLCH = 6           # lin1 weight stream chunks (even JPC so FWL pairs don't split)
JPC = NJ1 // NLCH  # 114 chunks per stream piece

_last_results = {"exec_time_ns": None}


def _ensure_axon_hooks():
    """Make BASS_TRACE-driven NTFF profiling under axon degrade gracefully."""
    try:
        import antenv.axon_hooks  # noqa: F401
        return
    except ImportError:
        pass
    import types
    try:
        import antenv
    except ImportError:
        return
    mod = types.ModuleType("antenv.axon_hooks")
    holder = {"hook": None}
    mod.set_axon_ntff_profile_hook = lambda h: holder.__setitem__("hook", h)
    mod.get_axon_ntff_profile_hook = lambda: holder["hook"]
    sys.modules["antenv.axon_hooks"] = mod
    antenv.axon_hooks = mod
    try:
        from trn_agent_boot.trn_boot import _ntff_profile_via_ctypes
        hook = _ntff_profile_via_ctypes('/opt/axon/libaxon_pjrt.so')
        if hook is not None:
            mod.set_axon_ntff_profile_hook(hook)
    except Exception:
        pass
    _orig_upload = bass_utils.upload_artifacts

    def _safe_upload(tmpdir):
        try:
            return _orig_upload(tmpdir)
        except Exception:
            return "local://" + str(tmpdir)

    bass_utils.upload_artifacts = _safe_upload


_ensure_axon_hooks()


def _build_program():
    nc = bacc.Bacc("TRN2", target_bir_lowering=False, debug=False)

    dt_in = {}

    def din(name, shape, dtype=F32):
        t = nc.dram_tensor(name, shape, dtype, kind="ExternalInput")
        dt_in[name] = t
        return t

    din("xb", [NH, 2 * NG], BF16)            # x chan-major [p, (fc, g, n)]
    din("mm", [NH, 2 * NG], BF16)            # multiplicity+I [p, (sc, g, d)]
    din("w1s", [NH, 2 * HC], BF16)           # W1 [p, (fc, 384)]
    din("w2s", [128, 3 * HC], BF16)          # W2 [p, (kc, 384)]
    din("was1", [NH, 2 * 12], BF16)          # W1@[as|ad] [p, (fc, 12)]
    din("was2", [128, 3 * 12], BF16)
    din("gncol", [128, 4], F32)              # graphnorm gamma, col ck
    din("gncol2", [128, 4], F32)
    din("lin1s", [128, NJ1 * C], BF16)       # lin1_w reordered (p, (ck, n, 64))
    din("dpati", [7, 6 * NG], BF16)          # block-diag head-selector pattern
    din("onesi", [1, NG], BF16)
    din("head64", [C, 4], F32)               # cols: lin1_b, bn_scale, bn_shift
    din("lin2w", [C, NCLS], F32)
    din("lin2b", [NCLS, 1], F32)

    out_d = nc.dram_tensor("out", [NCLS, GPC], F32, kind="ExternalOutput")

    with tile.TileContext(nc) as tc:
        _emit(tc, dt_in, out_d)

    nc.finalize()
    return nc


def _emit(tc, din, out_d):
    nc = tc.nc

    cst = tc.alloc_tile_pool(name="cst", bufs=1)
    lw = tc.alloc_tile_pool(name="lw", bufs=6)
    hp = tc.alloc_tile_pool(name="hp", bufs=1)
    att = tc.alloc_tile_pool(name="att", bufs=1)
    scp = tc.alloc_tile_pool(name="scp", bufs=2)
    agw = tc.alloc_tile_pool(name="agw", bufs=2)
    xo = tc.alloc_tile_pool(name="xo", bufs=1)
    wk = tc.alloc_tile_pool(name="wk", bufs=2)
    psH = tc.alloc_tile_pool(name="psH", bufs=1, space="PSUM")
    psS = tc.alloc_tile_pool(name="psS", bufs=1, space="PSUM")
    psZ = tc.alloc_tile_pool(name="psZ", bufs=2, space="PSUM")
    psN = tc.alloc_tile_pool(name="psN", bufs=2, space="PSUM")
    psT = tc.alloc_tile_pool(name="psT", bufs=1, space="PSUM")
    psY = tc.alloc_tile_pool(name="psY", bufs=1, space="PSUM")

    # ---- inputs: latency-critical tensors lead BOTH queues; the dpat
    # patterns and late constants follow; the big lin1 stream comes last ----
    identb = cst.tile([128, 128], BF16)
    masks.make_identity(nc, identb[:])

    dpats, a1os = [], []
    for lay in range(2):
        dp = cst.tile([7, 6 * NG], BF16, name=f"dpat{lay}")
        nc.sync.dma_start(dp[:], din["dpati"].ap()[:, :])
        ao = cst.tile([7, NG], BF16, name=f"a1o{lay}")
        nc.sync.dma_start(ao[6:7, :], din["onesi"].ap()[0:1, :])
        dpats.append(dp)
        a1os.append(ao)

    xb = cst.tile([NH, 2 * NG], BF16)
    nc.sync.dma_start(xb[:, 0:NG], din["xb"].ap()[:, 0:NG])
    nc.scalar.dma_start(xb[:, NG:2 * NG], din["xb"].ap()[:, NG:2 * NG])
    w1s = cst.tile([NH, 2 * HC], BF16)
    nc.sync.dma_start(w1s[:], din["w1s"].ap()[:, :])
    was1 = cst.tile([NH, 2 * 12], BF16)
    nc.sync.dma_start(was1[:], din["was1"].ap()[:, :])
    mmt = cst.tile([NH, 2 * NG], BF16)
    nc.sync.dma_start(mmt[:, 0:NG], din["mm"].ap()[:, 0:NG])
    nc.scalar.dma_start(mmt[:, NG:2 * NG], din["mm"].ap()[:, NG:2 * NG])

    w2s = cst.tile([128, 3 * HC], BF16)
    nc.sync.dma_start(w2s[:], din["w2s"].ap()[:, :])
    was2 = cst.tile([128, 3 * 12], BF16)
    nc.sync.dma_start(was2[:], din["was2"].ap()[:, :])
    gncol = cst.tile([128, 4], F32)
    nc.sync.dma_start(gncol[:], din["gncol"].ap()[:, :])
    gncol2 = cst.tile([128, 4], F32)
    nc.sync.dma_start(gncol2[:], din["gncol2"].ap()[:, :])
    head64 = cst.tile([C, 4], F32)
    nc.sync.dma_start(head64[:], din["head64"].ap()[:, :])
    lin2w = cst.tile([C, NCLS], F32)
    nc.sync.dma_start(lin2w[:], din["lin2w"].ap()[:, :])
    lin2b = cst.tile([NCLS, 1], F32)
    nc.sync.dma_start(lin2b[:], din["lin2b"].ap()[:, :])

    # lin1 weight stream, split across BOTH hardware DMA queues: chunks
    # 0/2/4/5 ride the Activation queue from the start; chunks 1/3 ride the
    # sync queue in the idle window between the two layers' dpat relocations.
    lin1_t = [None] * NLCH

    def lin1_chunk(i, eng):
        t = lw.tile([128, JPC * C], BF16, tag="lin1", name=f"lin1c{i}")
        eng.dma_start(t[:], din["lin1s"].ap()[:, i * JPC * C:(i + 1) * JPC * C])
        lin1_t[i] = t

    def emit_scalar_chunks():
        for i in (0, 2, 4, 5):
            lin1_chunk(i, nc.scalar)

    def layer(xBs, wts, wast, gcol, lay, hook_g1=None):
        """One GAT layer + elu + graphnorm for all 6 graphs.

        xBs: list of nkc channel-major input tiles [p, (g, n)] bf16.
        wts: [p, (kc, 384)] bf16; wast: [p, (kc, 12)] bf16.
        Returns one tile [128, (ck, g, n)] bf16 channel-major.

        Scores use exp(lrelu(a1+a2)) = max(exp(a1)exp(a2), exp(.2a1)exp(.2a2)):
        each product is rank-1 per head, so the dense [s, (h,d)] score tile is
        built by a single k=6 matmul against a block-diagonal exp(a2) operand
        instead of partition-broadcasts + dense scalar activations."""
        nkc = len(xBs)
        dpat = dpats[lay]
        a1o = a1os[lay]

        # attention logits a1/a2 = was.T @ x as separate [6, nb] matmuls so both
        # land at partition base 0
        a2T = att.tile([6, NG], BF16, tag="a2T")
        for nb in range(3):
            cols = slice(nb * 456, (nb + 1) * 456)
            a1_ps = psS.tile([6, 456], F32, tag="aps")
            for kc in range(nkc):
                nc.tensor.matmul(a1_ps[:], wast[:, kc * 12:kc * 12 + 6],
                                 xBs[kc][:, cols],
                                 start=(kc == 0), stop=(kc == nkc - 1))
            nc.vector.tensor_copy(a1o[0:6, cols], a1_ps[:])
            a2_ps = psS.tile([6, 456], F32, tag="aps")
            for kc in range(nkc):
                nc.tensor.matmul(a2_ps[:], wast[:, kc * 12 + 6:kc * 12 + 12],
                                 xBs[kc][:, cols],
                                 start=(kc == 0), stop=(kc == nkc - 1))
            nc.vector.tensor_copy(a2T[:, cols], a2_ps[:])
        # relocate a2 rows into dpat row 6 (partition shift), per nb chunk so
        # early graphs' score operands are ready before the last graph finishes
        for nb in range(3):
            for h in range(6):
                nc.sync.dma_start(
                    dpat[6:7, h * NG + nb * 456: h * NG + (nb + 1) * 456],
                    a2T[h:h + 1, nb * 456:(nb + 1) * 456])

        # h node-major [114, (sc, g, h, 65)] bf16 directly from channel-major
        # input; 65th col = 1 (ones written once per layer, disjoint cols)
        hA65 = hp.tile([NH, 2 * GPC * 390], BF16, tag=f"hA65{lay}")
        nc.gpsimd.memset(
            hA65[:].rearrange("p (b h c) -> p b h c", h=6, c=65)[:, :, :, 64:65], 1.0)
        for sc in range(2):
            for g in range(GPC):
                h_ps = psH.tile([NH, HC], F32, tag="hps")
                col0 = g * NPG + sc * NH
                for kc in range(nkc):
                    nc.tensor.matmul(h_ps[:], xBs[kc][:, col0:col0 + NH],
                                     wts[:, kc * HC:(kc + 1) * HC],
                                     start=(kc == 0), stop=(kc == nkc - 1))
                dst = hA65[:, (sc * GPC + g) * 390:(sc * GPC + g + 1) * 390] \
                    .rearrange("p (h c) -> p h c", c=65)
                nc.scalar.copy(dst[:, :, 0:64],
                               h_ps[:].rearrange("p (h c) -> p h c", h=6))

        # ---- attention + aggregation per graph ----
        xout = xo.tile([128, 3 * NG], BF16, tag=f"xn{lay}", name=f"xn{lay}")
        xeA = xo.tile([128, 3 * NG], BF16, tag="xeA")
        meanA = wk.tile([128, 3 * GPC], F32, tag="meanA")
        vepsA = wk.tile([128, 3 * GPC], F32, tag="vepsA")
        for g in range(GPC):
            if g == 1 and hook_g1 is not None:
                hook_g1()
            # dense scores zs[sc][s, (h, dpad 256)] = exp(lrelu(a1+a2)) * mult,
            # logits built in head-pair chunks by k=7 matmuls. Each head's d
            # dim is padded to 256 zero-backed columns so the aggregation
            # stationaries are full 128-column (FWL-eligible) loads.
            dv = dpat[:].rearrange("k (h g d) -> k h g d", g=GPC, d=NPG)
            zs = []
            for sc in range(2):
                zt = scp.tile([NH, 6 * 256], BF16, tag=f"z{sc}")
                ztv = zt[:].rearrange("p (h d) -> p h d", d=256)
                if lay == 0 and g < 2:
                    nc.vector.memset(ztv[:, :, NPG:256], 0.0)
                scol = g * NPG + sc * NH
                for hq in range(3):
                    e_ps = psZ.tile([NH, 2 * NPG], F32, tag="zz")
                    nc.tensor.matmul(e_ps[:], a1o[:, scol:scol + NH],
                                     dv[:, hq * 2:(hq + 1) * 2, g, :],
                                     start=True, stop=True)
                    zq = ztv[:, hq * 2:(hq + 1) * 2, 0:NPG]
                    eps2 = e_ps[:].rearrange("p (h d) -> p h d", h=2)
                    nc.scalar.activation(zq, eps2, Act.Prelu, alpha=0.2)
                    nc.scalar.activation(zq, zq, Act.Exp)
                    for hh in range(2):
                        h = hq * 2 + hh
                        zh = ztv[:, h, 0:NPG]
                        nc.vector.tensor_tensor(
                            out=zh, in0=zh,
                            in1=mmt[:, (sc * GPC + g) * NPG:(sc * GPC + g + 1) * NPG],
                            op=Alu.mult)
                zs.append(zt)

            # aggregation: psum [d, (h, 65)] per dc chunk (d 0:128 | 128:228);
            # col 64 = denominator. dc alternates between the two psN banks so
            # PSUM drains overlap; stationaries are 128-col FWL loads.
            dlen = (128, 100)
            x2p = agw.tile([128, 2 * HC], BF16, tag="x2p")
            n_ps = [psN.tile([128, 390], F32, tag="nps", name=f"nps{dd}") for dd in range(2)]
            for h in range(6):
                for dc in range(2):
                    for sc in range(2):
                        nc.tensor.matmul(
                            n_ps[dc][:, h * 65:(h + 1) * 65],
                            zs[sc][:, h * 256 + dc * 128: h * 256 + (dc + 1) * 128],
                            hA65[:, (sc * GPC + g) * 390 + h * 65:(sc * GPC + g) * 390 + (h + 1) * 65],
                            start=(sc == 0), stop=(sc == 1))
            for dc in range(2):
                dl = dlen[dc]
                rec = agw.tile([128, 6], F32, tag="rec")
                nc.vector.reciprocal(
                    rec[0:dl, :],
                    n_ps[dc][0:dl].rearrange("p (h c) -> p h c", c=65)[:, :, 64:65]
                    .rearrange("p h c -> p (h c)"))
                nc.vector.tensor_tensor(
                    out=x2p[0:dl, dc * HC:(dc + 1) * HC].rearrange("p (h c) -> p h c", h=6),
                    in0=n_ps[dc][0:dl].rearrange("p (h c) -> p h c", c=65)[:, :, 0:64],
                    in1=rec[0:dl].rearrange("p (h c) -> p h c", c=1).broadcast_to((dl, 6, 64)),
                    op=Alu.mult)
            # transpose this graph's columns to channel-major right away
            xg = wk.tile([128, 3 * NPG], BF16, tag="xg")
            for ck in range(3):
                tp = psT.tile([128, 2 * NH], BF16, tag="tp")
                nc.tensor.transpose(
                    tp[:, 0:128],
                    x2p[:, ck * 128:(ck + 1) * 128],
                    identb[:])
                nc.tensor.transpose(
                    tp[:, 128:228],
                    x2p[0:100, HC + ck * 128: HC + (ck + 1) * 128],
                    identb[0:100, 0:100])
                nc.scalar.copy(xg[:, ck * NPG:(ck + 1) * NPG], tp[:, 0:NPG])
            # elu over the whole graph at once, into the layer xe arena
            m = wk.tile([128, 3 * NPG], BF16, tag="m")
            nc.vector.tensor_scalar_min(m[:], xg[:], 0.0)
            nc.scalar.activation(m[:], m[:], Act.Exp)
            xe = xeA[:, g * 3 * NPG:(g + 1) * 3 * NPG]
            nc.vector.scalar_tensor_tensor(xe, m[:], -1.0, xg[:],
                                           op0=Alu.add, op1=Alu.max)
            # per-graph mean/var (DVE only; Ln/Exp batched to avoid activation
            # table swaps between exp and ln sets)
            xe3 = xe.rearrange("p (c n) -> p c n", c=3)
            s13 = wk.tile([128, 3], F32, tag="s13")
            nc.vector.tensor_reduce(s13[:], xe3, axis=mybir.AxisListType.X, op=Alu.add)
            sq = wk.tile([128, 3 * NPG], BF16, tag="sq")
            nc.vector.tensor_tensor(out=sq[:], in0=xe, in1=xe, op=Alu.mult)
            s23 = wk.tile([128, 3], F32, tag="s23")
            nc.vector.tensor_reduce(s23[:], sq[:].rearrange("p (c n) -> p c n", c=3),
                                    axis=mybir.AxisListType.X, op=Alu.add)
            mean3 = meanA[:, g * 3:(g + 1) * 3]
            nc.vector.tensor_scalar_mul(mean3, s13[:], 1.0 / NPG)
            msq3 = wk.tile([128, 3], F32, tag="msq3")
            nc.vector.tensor_tensor(out=msq3[:], in0=mean3, in1=mean3, op=Alu.mult)
            veps3 = vepsA[:, g * 3:(g + 1) * 3]
            nc.vector.scalar_tensor_tensor(veps3, s23[:], 1.0 / NPG, msq3[:],
                                           op0=Alu.mult, op1=Alu.subtract)
            nc.vector.tensor_scalar_add(veps3, veps3, 1e-5)

            def finish(gg):
                # out = xe * gisd - tcol   (gamma folded; beta==0)
                gisd3 = wk.tile([128, 3], F32, tag="gisd3")
                nc.vector.tensor_tensor(out=gisd3[:], in0=vepsA[:, gg * 3:(gg + 1) * 3],
                                        in1=gcol[:, 0:3], op=Alu.mult)
                tcol3 = wk.tile([128, 3], F32, tag="tcol3")
                nc.vector.tensor_tensor(out=tcol3[:], in0=meanA[:, gg * 3:(gg + 1) * 3],
                                        in1=gisd3[:], op=Alu.mult)
                for ck in range(3):
                    oc = xout[:, ck * NG + gg * NPG: ck * NG + (gg + 1) * NPG]
                    nc.vector.tensor_scalar_mul(
                        oc, xeA[:, gg * 3 * NPG + ck * NPG: gg * 3 * NPG + (ck + 1) * NPG],
                        gisd3[:, ck:ck + 1])
                    nc.vector.tensor_scalar_sub(oc, oc, tcol3[:, ck:ck + 1])

            if g == GPC - 2 or g == GPC - 1:
                # one Ln/Exp for graphs [0..4] at g==4, then [5] at g==5:
                # overwrite veps in place with 1/sqrt(veps)
                lo = 0 if g == GPC - 2 else (GPC - 1) * 3
                hi = (GPC - 1) * 3 if g == GPC - 2 else GPC * 3
                nc.scalar.activation(vepsA[:, lo:hi], vepsA[:, lo:hi], Act.Ln)
                nc.scalar.activation(vepsA[:, lo:hi], vepsA[:, lo:hi], Act.Exp,
                                     scale=-0.5)
                for gg in range(lo // 3, hi // 3):
                    finish(gg)
        return xout

    x2 = layer([xb[:, 0:NG], xb[:, NG:2 * NG]], w1s, was1, gncol, 0,
               hook_g1=emit_scalar_chunks)
    x2v = [x2[:, ck * NG:(ck + 1) * NG] for ck in range(3)]
    lin1_chunk(1, nc.sync)
    lin1_chunk(3, nc.sync)
    x3t = layer(x2v, w2s, was2, gncol2, 1)
    x3 = [x3t[:, ck * NG:(ck + 1) * NG] for ck in range(3)]

    # ---- lin1 GEMV: 684 k=128 chunks processed in PAIRS. Each pair loads a
    # full [128, 128] stationary tile (two adjacent nodes' weight chunks side
    # by side -> FWL-eligible) against a [128, 2, 6] moving slice. Diagonal
    # blocks of the [128, 12] psum hold the real partials; off-diagonal blocks
    # accumulate ignored cross terms. ----
    y_ps = psY.tile([128, 2 * GPC], F32, tag="y")
    for i in range(NLCH):
        lt = lin1_t[i]
        for jj in range(0, JPC, 2):
            jc = i * JPC + jj
            ck, n = jc // NPG, jc % NPG
            x3r = x3[ck].rearrange("p (g n) -> p n g", g=GPC)
            nc.tensor.matmul(y_ps[:], lt[:, jj * C:(jj + 2) * C],
                             x3r[:, n:n + 2, :],
                             start=(jc == 0), stop=(jc == NJ1 - 2))

    # fold: y = y_ps[0:64, 0:6] + y_ps[64:128, 6:12] (partition shift via DMA)
    yhi = wk.tile([128, GPC], F32, tag="yhi")
    nc.scalar.copy(yhi[64:128, :], y_ps[64:128, GPC:2 * GPC])
    ylo = wk.tile([C, GPC], F32, tag="ylo")
    nc.sync.dma_start(ylo[:], yhi[64:128, :])

    # ---- head: +b, elu, bn, lin2 ----
    yb = wk.tile([C, GPC], F32, tag="yb")
    nc.vector.scalar_tensor_tensor(yb[:], y_ps[0:C, 0:GPC], head64[:, 0:1],
                                   ylo[:], op0=Alu.add, op1=Alu.add)
    m2 = wk.tile([C, GPC], F32, tag="m2")
    nc.vector.tensor_scalar_min(m2[:], yb[:], 0.0)
    e2 = wk.tile([C, GPC], F32, tag="e2")
    nc.scalar.activation(e2[:], m2[:], Act.Exp)
    ye = wk.tile([C, GPC], F32, tag="ye")
    nc.vector.scalar_tensor_tensor(ye[:], e2[:], -1.0, yb[:], op0=Alu.add, op1=Alu.max)
    yn = wk.tile([C, GPC], F32, tag="yn")
    nc.vector.scalar_tensor_tensor(yn[:], ye[:], head64[:, 1:2],
                                   head64[:, 2:3].broadcast_to((C, GPC)),
                                   op0=Alu.mult, op1=Alu.add)
    o_ps = psY.tile([128, 2 * GPC], F32, tag="y")
    nc.tensor.matmul(o_ps[0:NCLS, 0:GPC], lin2w[:], yn[:], start=True, stop=True)
    ob = wk.tile([NCLS, GPC], F32, tag="ob")
    nc.vector.tensor_scalar_add(ob[:], o_ps[0:NCLS, 0:GPC], lin2b[:])
    nc.sync.dma_start(out_d.ap()[:, :], ob[:])

    for p in (psY, psT, psN, psZ, psS, psH, wk, xo, agw, scp, att, hp, lw, cst):
        p.release()


def _host_prep(inputs):
    """Build per-core input maps (sharding / relayout / dtype prep)."""
    import ml_dtypes
    x = np.asarray(inputs["x"], np.float32)
    ei = np.asarray(inputs["edge_index"])
    src, dst = np.asarray(ei[0], np.int64), np.asarray(ei[1], np.int64)

    # multiplicity matrices M[g, s, d] (+ self loops)
    g_of = src // NPG
    sl = src - g_of * NPG
    dl = dst - (dst // NPG) * NPG
    flat = g_of * (NPG * NPG) + sl * NPG + dl
    Mall = np.bincount(flat, minlength=B * NPG * NPG).astype(np.float32).reshape(B, NPG, NPG)
    Mall[:, np.arange(NPG), np.arange(NPG)] += 1.0

    xg = x.reshape(B, NPG, F_IN)

    def mk_asad(a_s, a_d):
        a_s = np.asarray(a_s, np.float32)
        a_d = np.asarray(a_d, np.float32)
        out = np.zeros((HC, 12), np.float32)
        for h in range(H):
            out[h * C:(h + 1) * C, h] = a_s[h]
            out[h * C:(h + 1) * C, 6 + h] = a_d[h]
        return out

    w1 = np.asarray(inputs["w1"], np.float32)
    w2 = np.asarray(inputs["w2"], np.float32)
    was1 = w1 @ mk_asad(inputs["as1"], inputs["ad1"])   # [228, 12]
    was2 = w2 @ mk_asad(inputs["as2"], inputs["ad2"])   # [384, 12]

    # kernel folds assume zero biases / unit mean-scale (true for this model)
    for nm in ("b1", "b2", "gn1_b", "gn2_b"):
        assert np.abs(np.asarray(inputs[nm])).max() == 0.0, f"{nm} nonzero"
    for nm in ("gn1_ms", "gn2_ms"):
        assert np.abs(np.asarray(inputs[nm]) - 1.0).max() == 0.0, f"{nm} != 1"

    bn_w = np.asarray(inputs["bn_w"], np.float64)
    bn_b = np.asarray(inputs["bn_b"], np.float64)
    bn_rm = np.asarray(inputs["bn_rm"], np.float64)
    bn_rv = np.asarray(inputs["bn_rv"], np.float64)
    bn_sc = bn_w / np.sqrt(bn_rv + 1e-5)
    bn_sh = bn_b - bn_rm * bn_sc
    head64 = np.stack([np.asarray(inputs["lin1_b"], np.float64),
                       bn_sc, bn_sh, np.zeros((C,))], axis=1).astype(np.float32)

    # lin1 reorder: rows j=(n*384 + ck*128 + p) -> chunks (ck, n) of k=128
    lwt = np.asarray(inputs["lin1_w"], np.float32).reshape(NPG, 3, 128, C)
    lin1s = np.ascontiguousarray(lwt.transpose(2, 1, 0, 3)).reshape(128, NJ1 * C) \
        .astype(ml_dtypes.bfloat16)

    def cm(a):
        """[g, n, f] -> [114 (f-part), (fc, g, n)] bf16 channel-major."""
        gg, nn, ff = a.shape
        nkc = ff // NH
        t = a.transpose(2, 0, 1).reshape(nkc, NH, gg, nn).transpose(1, 0, 2, 3)
        return np.ascontiguousarray(t).reshape(NH, nkc * gg * nn).astype(ml_dtypes.bfloat16)

    gnc1 = np.zeros((128, 4), np.float32)
    gnc2 = np.zeros((128, 4), np.float32)
    gnc1[:, 0:3] = np.asarray(inputs["gn1_w"], np.float32).reshape(3, 128).T
    gnc2[:, 0:3] = np.asarray(inputs["gn2_w"], np.float32).reshape(3, 128).T

    dpati = np.zeros((7, 6 * NG), np.float32)
    for j in range(7 - 1):
        dpati[j, j * NG:(j + 1) * NG] = 1.0
    shared = dict(
        dpati=dpati.astype(ml_dtypes.bfloat16),
        onesi=np.ones((1, NG), ml_dtypes.bfloat16),
        w1s=np.ascontiguousarray(
            w1.reshape(2, NH, HC).transpose(1, 0, 2)).reshape(NH, 2 * HC)
            .astype(ml_dtypes.bfloat16),
        w2s=np.ascontiguousarray(
            w2.reshape(3, 128, HC).transpose(1, 0, 2)).reshape(128, 3 * HC)
            .astype(ml_dtypes.bfloat16),
        was1=np.ascontiguousarray(
            was1.reshape(2, NH, 12).transpose(1, 0, 2)).reshape(NH, 24)
            .astype(ml_dtypes.bfloat16),
        was2=np.ascontiguousarray(
            was2.reshape(3, 128, 12).transpose(1, 0, 2)).reshape(128, 36)
            .astype(ml_dtypes.bfloat16),
        gncol=gnc1, gncol2=gnc2,
        lin1s=lin1s, head64=head64,
        lin2w=np.asarray(inputs["lin2_w"], np.float32),
        lin2b=np.asarray(inputs["lin2_b"], np.float32).reshape(NCLS, 1),
    )

    in_maps = []
    for core in range(NCORES):
        gs = slice(core * GPC, (core + 1) * GPC)
        m = dict(shared)
        m["xb"] = cm(xg[gs])                           # [114, (fc, g, n)]
        m["mm"] = cm(Mall[gs].transpose(0, 2, 1))      # [114 (s), (sc, g, d)]
        in_maps.append(m)
    return in_maps


_cached_nc = None


def kernel(**inputs):
    global _cached_nc
    in_maps = _host_prep(inputs)
    if _cached_nc is None:
        _cached_nc = _build_program()
    nc = _cached_nc
    res = bass_utils.run_bass_kernel_spmd(nc, in_maps, core_ids=list(range(NCORES)))
    _last_results["exec_time_ns"] = res.exec_time_ns
    _last_results["res"] = res
    out = np.zeros((B, NCLS), np.float32)
    for core in range(NCORES):
        o = res.results[core]["out"]          # [2, 6]
        out[core * GPC:(core + 1) * GPC, :] = o.T
    return out



# revision 39
# speedup vs baseline: 1.0563x; 1.0563x over previous
"""GAT (2-layer, 6-head) + GraphNorm + readout MLP on 8 Trainium2 cores.

Sharding: graph-level data parallelism. 48 fixed-size graphs (228 nodes,
edges never cross graphs) -> 6 graphs per core. Weights replicated.

v2 redesign vs the per-graph baseline:
  - All 6 graphs batched per stage; channel-major [c, (g, n)] primary layout.
  - Dense attention scores z[s,d] built with GpSimd partition-broadcast of
    the a2 rows + one wide DVE add per graph ([114, 2*6*228] bf16 tiles,
    both source-halves per instruction), lrelu on DVE, exp on Scalar
    (single activation table: exp/ln/relu/copy), multiplicity mask on GpSimd.
  - Attention logits a1/a2 computed straight from the layer input with
    host-folded was = W @ [as|ad].
  - Aggregation: dest-partition matmuls with a fused ones-column so the
    softmax denominator falls out of the same PSUM tile.
  - lin1 readout: weights host-reordered to (ck, n, p) chunks of k=128 so
    the GEMV consumes the channel-major layer-2 output directly; weights
    streamed in 4 double-buffered DMA chunks overlapping the layer phase.

kernel(**inputs) -> np.ndarray [48, 2] float32.
"""
import sys
sys.path.insert(0, '/opt/trn_rl_repo')

import numpy as np

import concourse.bass as bass
import concourse.bacc as bacc
import concourse.mybir as mybir
import concourse.tile as tile
from concourse import masks
from concourse import bass_utils

F32 = mybir.dt.float32
BF16 = mybir.dt.bfloat16
Alu = mybir.AluOpType
Act = mybir.ActivationFunctionType

H, C = 6, 64
HC = 384
NPG = 228          # nodes per graph
B = 48             # graphs
GPC = 6            # graphs per core
NCORES = 8
F_IN = 228
NH = 114           # node half-chunk
NCLS = 2
NG = GPC * NPG     # 1368 node-columns per core
NJ1 = 3 * NPG      # 684 lin1 k-chunks of 128
NLCH = 6           # lin1 weight stream chunks (even JPC so FWL pairs don't split)
JPC = NJ1 // NLCH  # 114 chunks per stream piece

_last_results = {"exec_time_ns": None}


def _ensure_axon_hooks():
    """Make BASS_TRACE-driven NTFF profiling under axon degrade gracefully."""
    try:
        import antenv.axon_hooks  # noqa: F401
        return
    except ImportError:
        pass
    import types
    try:
        import antenv
    except ImportError:
        return
    mod = types.ModuleType("antenv.axon_hooks")
    holder = {"hook": None}
    mod.set_axon_ntff_profile_hook = lambda h: holder.__setitem__("hook", h)
    mod.get_axon_ntff_profile_hook = lambda: holder["hook"]
    sys.modules["antenv.axon_hooks"] = mod
    antenv.axon_hooks = mod
    try:
        from trn_agent_boot.trn_boot import _ntff_profile_via_ctypes
        hook = _ntff_profile_via_ctypes('/opt/axon/libaxon_pjrt.so')
        if hook is not None:
            mod.set_axon_ntff_profile_hook(hook)
    except Exception:
        pass
    _orig_upload = bass_utils.upload_artifacts

    def _safe_upload(tmpdir):
        try:
            return _orig_upload(tmpdir)
        except Exception:
            return "local://" + str(tmpdir)

    bass_utils.upload_artifacts = _safe_upload


_ensure_axon_hooks()


def _build_program():
    nc = bacc.Bacc("TRN2", target_bir_lowering=False, debug=False)

    dt_in = {}

    def din(name, shape, dtype=F32):
        t = nc.dram_tensor(name, shape, dtype, kind="ExternalInput")
        dt_in[name] = t
        return t

    din("xb", [NH, 2 * NG], BF16)            # x chan-major [p, (fc, g, n)]
    din("mm", [NH, 2 * NG], BF16)            # multiplicity+I [p, (sc, g, d)]
    din("w1s", [NH, 2 * HC], BF16)           # W1 [p, (fc, 384)]
    din("w2s", [128, 3 * HC], BF16)          # W2 [p, (kc, 384)]
    din("was1", [NH, 2 * 12], BF16)          # W1@[as|ad] [p, (fc, 12)]
    din("was2", [128, 3 * 12], BF16)
    din("gncol", [128, 4], F32)              # graphnorm gamma, col ck
    din("gncol2", [128, 4], F32)
    din("lin1s", [128, NJ1 * C], BF16)       # lin1_w reordered (p, (ck, n, 64))
    din("dpati", [7, 6 * NG], BF16)          # block-diag head-selector pattern
    din("onesi", [1, NG], BF16)
    din("head64", [C, 4], F32)               # cols: lin1_b, bn_scale, bn_shift
    din("lin2w", [C, NCLS], F32)
    din("lin2b", [NCLS, 1], F32)

    out_d = nc.dram_tensor("out", [NCLS, GPC], F32, kind="ExternalOutput")

    with tile.TileContext(nc) as tc:
        _emit(tc, dt_in, out_d)

    nc.finalize()
    return nc


def _emit(tc, din, out_d):
    nc = tc.nc

    cst = tc.alloc_tile_pool(name="cst", bufs=1)
    lw = tc.alloc_tile_pool(name="lw", bufs=6)
    hp = tc.alloc_tile_pool(name="hp", bufs=1)
    att = tc.alloc_tile_pool(name="att", bufs=1)
    scp = tc.alloc_tile_pool(name="scp", bufs=2)
    agw = tc.alloc_tile_pool(name="agw", bufs=2)
    xo = tc.alloc_tile_pool(name="xo", bufs=1)
    wk = tc.alloc_tile_pool(name="wk", bufs=2)
    psH = tc.alloc_tile_pool(name="psH", bufs=1, space="PSUM")
    psS = tc.alloc_tile_pool(name="psS", bufs=1, space="PSUM")
    psZ = tc.alloc_tile_pool(name="psZ", bufs=2, space="PSUM")
    psN = tc.alloc_tile_pool(name="psN", bufs=2, space="PSUM")
    psT = tc.alloc_tile_pool(name="psT", bufs=1, space="PSUM")
    psY = tc.alloc_tile_pool(name="psY", bufs=1, space="PSUM")

    # ---- inputs: latency-critical tensors lead BOTH queues; the dpat
    # patterns and late constants follow; the big lin1 stream comes last ----
    identb = cst.tile([128, 128], BF16)
    masks.make_identity(nc, identb[:])

    dpats, a1os = [], []
    for lay in range(2):
        dp = cst.tile([7, 6 * NG], BF16, name=f"dpat{lay}")
        nc.sync.dma_start(dp[:], din["dpati"].ap()[:, :])
        ao = cst.tile([7, NG], BF16, name=f"a1o{lay}")
        nc.sync.dma_start(ao[6:7, :], din["onesi"].ap()[0:1, :])
        dpats.append(dp)
        a1os.append(ao)

    xb = cst.tile([NH, 2 * NG], BF16)
    nc.sync.dma_start(xb[:, 0:NG], din["xb"].ap()[:, 0:NG])
    nc.scalar.dma_start(xb[:, NG:2 * NG], din["xb"].ap()[:, NG:2 * NG])
    w1s = cst.tile([NH, 2 * HC], BF16)
    nc.sync.dma_start(w1s[:], din["w1s"].ap()[:, :])
    was1 = cst.tile([NH, 2 * 12], BF16)
    nc.sync.dma_start(was1[:], din["was1"].ap()[:, :])
    mmt = cst.tile([NH, 2 * NG], BF16)
    nc.sync.dma_start(mmt[:, 0:NG], din["mm"].ap()[:, 0:NG])
    nc.scalar.dma_start(mmt[:, NG:2 * NG], din["mm"].ap()[:, NG:2 * NG])

    w2s = cst.tile([128, 3 * HC], BF16)
    nc.sync.dma_start(w2s[:], din["w2s"].ap()[:, :])
    was2 = cst.tile([128, 3 * 12], BF16)
    nc.sync.dma_start(was2[:], din["was2"].ap()[:, :])
    gncol = cst.tile([128, 4], F32)
    nc.sync.dma_start(gncol[:], din["gncol"].ap()[:, :])
    gncol2 = cst.tile([128, 4], F32)
    nc.sync.dma_start(gncol2[:], din["gncol2"].ap()[:, :])
    head64 = cst.tile([C, 4], F32)
    nc.sync.dma_start(head64[:], din["head64"].ap()[:, :])
    lin2w = cst.tile([C, NCLS], F32)
    nc.sync.dma_start(lin2w[:], din["lin2w"].ap()[:, :])
    lin2b = cst.tile([NCLS, 1], F32)
    nc.sync.dma_start(lin2b[:], din["lin2b"].ap()[:, :])

    # lin1 weight stream, split across BOTH hardware DMA queues: chunks
    # 0/2/4/5 ride the Activation queue from the start; chunks 1/3 ride the
    # sync queue in the idle window between the two layers' dpat relocations.
    lin1_t = [None] * NLCH

    def lin1_chunk(i, eng, gate=None):
        t = lw.tile([128, JPC * C], BF16, tag="lin1", name=f"lin1c{i}")
        if gate is not None:
            # WAW gate on the idle Scalar engine: holds the big stream back
            # until the critical input DMAs drain (shared DMA engines would
            # otherwise starve them)
            nc.scalar.copy(t[0:1, 0:1], gate)
        eng.dma_start(t[:], din["lin1s"].ap()[:, i * JPC * C:(i + 1) * JPC * C])
        lin1_t[i] = t

    def emit_scalar_chunks():
        for i in (0, 2, 4, 5):
            lin1_chunk(i, nc.scalar, gate=mmt[0:1, NG - 1:NG])

    def layer(xBs, wts, wast, gcol, lay, hook_g1=None):
        """One GAT layer + elu + graphnorm for all 6 graphs.

        xBs: list of nkc channel-major input tiles [p, (g, n)] bf16.
        wts: [p, (kc, 384)] bf16; wast: [p, (kc, 12)] bf16.
        Returns one tile [128, (ck, g, n)] bf16 channel-major.

        Scores use exp(lrelu(a1+a2)) = max(exp(a1)exp(a2), exp(.2a1)exp(.2a2)):
        each product is rank-1 per head, so the dense [s, (h,d)] score tile is
        built by a single k=6 matmul against a block-diagonal exp(a2) operand
        instead of partition-broadcasts + dense scalar activations."""
        nkc = len(xBs)
        dpat = dpats[lay]
        a1o = a1os[lay]

        # attention logits a1/a2 = was.T @ x as separate [6, nb] matmuls so both
        # land at partition base 0
        a2T = att.tile([6, NG], BF16, tag="a2T")
        for nb in range(3):
            cols = slice(nb * 456, (nb + 1) * 456)
            a1_ps = psS.tile([6, 456], F32, tag="aps")
            for kc in range(nkc):
                nc.tensor.matmul(a1_ps[:], wast[:, kc * 12:kc * 12 + 6],
                                 xBs[kc][:, cols],
                                 start=(kc == 0), stop=(kc == nkc - 1))
            nc.vector.tensor_copy(a1o[0:6, cols], a1_ps[:])
            a2_ps = psS.tile([6, 456], F32, tag="aps")
            for kc in range(nkc):
                nc.tensor.matmul(a2_ps[:], wast[:, kc * 12 + 6:kc * 12 + 12],
                                 xBs[kc][:, cols],
                                 start=(kc == 0), stop=(kc == nkc - 1))
            nc.vector.tensor_copy(a2T[:, cols], a2_ps[:])
        # relocate a2 rows into dpat row 6 (partition shift), per nb chunk so
        # early graphs' score operands are ready before the last graph finishes
        for nb in range(3):
            for h in range(6):
                nc.sync.dma_start(
                    dpat[6:7, h * NG + nb * 456: h * NG + (nb + 1) * 456],
                    a2T[h:h + 1, nb * 456:(nb + 1) * 456])

        # h node-major [114, (sc, g, h, 65)] bf16 directly from channel-major
        # input; 65th col = 1 (ones written once per layer, disjoint cols)
        hA65 = hp.tile([NH, 2 * GPC * 390], BF16, tag=f"hA65{lay}")
        nc.gpsimd.memset(
            hA65[:].rearrange("p (b h c) -> p b h c", h=6, c=65)[:, :, :, 64:65], 1.0)
        for sc in range(2):
            for g in range(GPC):
                h_ps = psH.tile([NH, HC], F32, tag="hps")
                col0 = g * NPG + sc * NH
                for kc in range(nkc):
                    nc.tensor.matmul(h_ps[:], xBs[kc][:, col0:col0 + NH],
                                     wts[:, kc * HC:(kc + 1) * HC],
                                     start=(kc == 0), stop=(kc == nkc - 1))
                dst = hA65[:, (sc * GPC + g) * 390:(sc * GPC + g + 1) * 390] \
                    .rearrange("p (h c) -> p h c", c=65)
                nc.scalar.copy(dst[:, :, 0:64],
                               h_ps[:].rearrange("p (h c) -> p h c", h=6))

        # ---- attention + aggregation per graph ----
        xout = xo.tile([128, 3 * NG], BF16, tag=f"xn{lay}", name=f"xn{lay}")
        xeA = xo.tile([128, 3 * NG], BF16, tag="xeA")
        meanA = wk.tile([128, 3 * GPC], F32, tag="meanA")
        vepsA = wk.tile([128, 3 * GPC], F32, tag="vepsA")
        for g in range(GPC):
            if g == 1 and hook_g1 is not None:
                hook_g1()
            # dense scores zs[sc][s, (h, dpad 256)] = exp(lrelu(a1+a2)) * mult,
            # logits built in head-pair chunks by k=7 matmuls. Each head's d
            # dim is padded to 256 zero-backed columns so the aggregation
            # stationaries are full 128-column (FWL-eligible) loads.
            dv = dpat[:].rearrange("k (h g d) -> k h g d", g=GPC, d=NPG)
            zs = []
            for sc in range(2):
                zt = scp.tile([NH, 6 * 256], BF16, tag=f"z{sc}")
                ztv = zt[:].rearrange("p (h d) -> p h d", d=256)
                if lay == 0 and g < 2:
                    nc.vector.memset(ztv[:, :, NPG:256], 0.0)
                scol = g * NPG + sc * NH
                for hq in range(3):
                    e_ps = psZ.tile([NH, 2 * NPG], F32, tag="zz")
                    nc.tensor.matmul(e_ps[:], a1o[:, scol:scol + NH],
                                     dv[:, hq * 2:(hq + 1) * 2, g, :],
                                     start=True, stop=True)
                    zq = ztv[:, hq * 2:(hq + 1) * 2, 0:NPG]
                    eps2 = e_ps[:].rearrange("p (h d) -> p h d", h=2)
                    nc.scalar.activation(zq, eps2, Act.Prelu, alpha=0.2)
                    nc.scalar.activation(zq, zq, Act.Exp)
                    for hh in range(2):
                        h = hq * 2 + hh
                        zh = ztv[:, h, 0:NPG]
                        nc.vector.tensor_tensor(
                            out=zh, in0=zh,
                            in1=mmt[:, (sc * GPC + g) * NPG:(sc * GPC + g + 1) * NPG],
                            op=Alu.mult)
                zs.append(zt)

            # aggregation: psum [d, (h, 65)] per dc chunk (d 0:128 | 128:228);
            # col 64 = denominator. dc alternates between the two psN banks so
            # PSUM drains overlap; stationaries are 128-col FWL loads.
            dlen = (128, 100)
            x2p = agw.tile([128, 2 * HC], BF16, tag="x2p")
            n_ps = [psN.tile([128, 390], F32, tag="nps", name=f"nps{dd}") for dd in range(2)]
            for h in range(6):
                for dc in range(2):
                    for sc in range(2):
                        nc.tensor.matmul(
                            n_ps[dc][:, h * 65:(h + 1) * 65],
                            zs[sc][:, h * 256 + dc * 128: h * 256 + (dc + 1) * 128],
                            hA65[:, (sc * GPC + g) * 390 + h * 65:(sc * GPC + g) * 390 + (h + 1) * 65],
                            start=(sc == 0), stop=(sc == 1))
            for dc in range(2):
                dl = dlen[dc]
                rec = agw.tile([128, 6], F32, tag="rec")
                nc.vector.reciprocal(
                    rec[0:dl, :],
                    n_ps[dc][0:dl].rearrange("p (h c) -> p h c", c=65)[:, :, 64:65]
                    .rearrange("p h c -> p (h c)"))
                nc.vector.tensor_tensor(
                    out=x2p[0:dl, dc * HC:(dc + 1) * HC].rearrange("p (h c) -> p h c", h=6),
                    in0=n_ps[dc][0:dl].rearrange("p (h c) -> p h c", c=65)[:, :, 0:64],
                    in1=rec[0:dl].rearrange("p (h c) -> p h c", c=1).broadcast_to((dl, 6, 64)),
                    op=Alu.mult)
            # transpose this graph's columns to channel-major right away
            xg = wk.tile([128, 3 * NPG], BF16, tag="xg")
            for ck in range(3):
                tp = psT.tile([128, 2 * NH], BF16, tag="tp")
                nc.tensor.transpose(
                    tp[:, 0:128],
                    x2p[:, ck * 128:(ck + 1) * 128],
                    identb[:])
                nc.tensor.transpose(
                    tp[:, 128:228],
                    x2p[0:100, HC + ck * 128: HC + (ck + 1) * 128],
                    identb[0:100, 0:100])
                nc.scalar.copy(xg[:, ck * NPG:(ck + 1) * NPG], tp[:, 0:NPG])
            # elu over the whole graph at once, into the layer xe arena
            m = wk.tile([128, 3 * NPG], BF16, tag="m")
            nc.vector.tensor_scalar_min(m[:], xg[:], 0.0)
            nc.scalar.activation(m[:], m[:], Act.Exp)
            xe = xeA[:, g * 3 * NPG:(g + 1) * 3 * NPG]
            nc.vector.scalar_tensor_tensor(xe, m[:], -1.0, xg[:],
                                           op0=Alu.add, op1=Alu.max)
            # per-graph mean/var (DVE only; Ln/Exp batched to avoid activation
            # table swaps between exp and ln sets)
            xe3 = xe.rearrange("p (c n) -> p c n", c=3)
            s13 = wk.tile([128, 3], F32, tag="s13")
            nc.vector.tensor_reduce(s13[:], xe3, axis=mybir.AxisListType.X, op=Alu.add)
            sq = wk.tile([128, 3 * NPG], BF16, tag="sq")
            nc.vector.tensor_tensor(out=sq[:], in0=xe, in1=xe, op=Alu.mult)
            s23 = wk.tile([128, 3], F32, tag="s23")
            nc.vector.tensor_reduce(s23[:], sq[:].rearrange("p (c n) -> p c n", c=3),
                                    axis=mybir.AxisListType.X, op=Alu.add)
            mean3 = meanA[:, g * 3:(g + 1) * 3]
            nc.vector.tensor_scalar_mul(mean3, s13[:], 1.0 / NPG)
            msq3 = wk.tile([128, 3], F32, tag="msq3")
            nc.vector.tensor_tensor(out=msq3[:], in0=mean3, in1=mean3, op=Alu.mult)
            veps3 = vepsA[:, g * 3:(g + 1) * 3]
            nc.vector.scalar_tensor_tensor(veps3, s23[:], 1.0 / NPG, msq3[:],
                                           op0=Alu.mult, op1=Alu.subtract)
            nc.vector.tensor_scalar_add(veps3, veps3, 1e-5)

            def finish(gg):
                # out = xe * gisd - tcol   (gamma folded; beta==0)
                gisd3 = wk.tile([128, 3], F32, tag="gisd3")
                nc.vector.tensor_tensor(out=gisd3[:], in0=vepsA[:, gg * 3:(gg + 1) * 3],
                                        in1=gcol[:, 0:3], op=Alu.mult)
                tcol3 = wk.tile([128, 3], F32, tag="tcol3")
                nc.vector.tensor_tensor(out=tcol3[:], in0=meanA[:, gg * 3:(gg + 1) * 3],
                                        in1=gisd3[:], op=Alu.mult)
                for ck in range(3):
                    oc = xout[:, ck * NG + gg * NPG: ck * NG + (gg + 1) * NPG]
                    nc.vector.tensor_scalar_mul(
                        oc, xeA[:, gg * 3 * NPG + ck * NPG: gg * 3 * NPG + (ck + 1) * NPG],
                        gisd3[:, ck:ck + 1])
                    nc.vector.tensor_scalar_sub(oc, oc, tcol3[:, ck:ck + 1])

            if g == GPC - 2 or g == GPC - 1:
                # one Ln/Exp for graphs [0..4] at g==4, then [5] at g==5:
                # overwrite veps in place with 1/sqrt(veps)
                lo = 0 if g == GPC - 2 else (GPC - 1) * 3
                hi = (GPC - 1) * 3 if g == GPC - 2 else GPC * 3
                nc.scalar.activation(vepsA[:, lo:hi], vepsA[:, lo:hi], Act.Ln)
                nc.scalar.activation(vepsA[:, lo:hi], vepsA[:, lo:hi], Act.Exp,
                                     scale=-0.5)
                for gg in range(lo // 3, hi // 3):
                    finish(gg)
        return xout

    x2 = layer([xb[:, 0:NG], xb[:, NG:2 * NG]], w1s, was1, gncol, 0,
               hook_g1=emit_scalar_chunks)
    x2v = [x2[:, ck * NG:(ck + 1) * NG] for ck in range(3)]
    lin1_chunk(1, nc.sync)
    lin1_chunk(3, nc.sync)
    x3t = layer(x2v, w2s, was2, gncol2, 1)
    x3 = [x3t[:, ck * NG:(ck + 1) * NG] for ck in range(3)]

    # ---- lin1 GEMV: 684 k=128 chunks processed in PAIRS. Each pair loads a
    # full [128, 128] stationary tile (two adjacent nodes' weight chunks side
    # by side -> FWL-eligible) against a [128, 2, 6] moving slice. Diagonal
    # blocks of the [128, 12] psum hold the real partials; off-diagonal blocks
    # accumulate ignored cross terms. ----
    y_ps = psY.tile([128, 2 * GPC], F32, tag="y")
    for i in range(NLCH):
        lt = lin1_t[i]
        for jj in range(0, JPC, 2):
            jc = i * JPC + jj
            ck, n = jc // NPG, jc % NPG
            x3r = x3[ck].rearrange("p (g n) -> p n g", g=GPC)
            nc.tensor.matmul(y_ps[:], lt[:, jj * C:(jj + 2) * C],
                             x3r[:, n:n + 2, :],
                             start=(jc == 0), stop=(jc == NJ1 - 2))

    # fold: y = y_ps[0:64, 0:6] + y_ps[64:128, 6:12] (partition shift via DMA)
    yhi = wk.tile([128, GPC], F32, tag="yhi")
    nc.scalar.copy(yhi[64:128, :], y_ps[64:128, GPC:2 * GPC])
    ylo = wk.tile([C, GPC], F32, tag="ylo")
    nc.sync.dma_start(ylo[:], yhi[64:128, :])

    # ---- head: +b, elu, bn, lin2 ----
    yb = wk.tile([C, GPC], F32, tag="yb")
    nc.vector.scalar_tensor_tensor(yb[:], y_ps[0:C, 0:GPC], head64[:, 0:1],
                                   ylo[:], op0=Alu.add, op1=Alu.add)
    m2 = wk.tile([C, GPC], F32, tag="m2")
    nc.vector.tensor_scalar_min(m2[:], yb[:], 0.0)
    e2 = wk.tile([C, GPC], F32, tag="e2")
    nc.scalar.activation(e2[:], m2[:], Act.Exp)
    ye = wk.tile([C, GPC], F32, tag="ye")
    nc.vector.scalar_tensor_tensor(ye[:], e2[:], -1.0, yb[:], op0=Alu.add, op1=Alu.max)
    yn = wk.tile([C, GPC], F32, tag="yn")
    nc.vector.scalar_tensor_tensor(yn[:], ye[:], head64[:, 1:2],
                                   head64[:, 2:3].broadcast_to((C, GPC)),
                                   op0=Alu.mult, op1=Alu.add)
    o_ps = psY.tile([128, 2 * GPC], F32, tag="y")
    nc.tensor.matmul(o_ps[0:NCLS, 0:GPC], lin2w[:], yn[:], start=True, stop=True)
    ob = wk.tile([NCLS, GPC], F32, tag="ob")
    nc.vector.tensor_scalar_add(ob[:], o_ps[0:NCLS, 0:GPC], lin2b[:])
    nc.sync.dma_start(out_d.ap()[:, :], ob[:])

    for p in (psY, psT, psN, psZ, psS, psH, wk, xo, agw, scp, att, hp, lw, cst):
        p.release()


def _host_prep(inputs):
    """Build per-core input maps (sharding / relayout / dtype prep)."""
    import ml_dtypes
    x = np.asarray(inputs["x"], np.float32)
    ei = np.asarray(inputs["edge_index"])
    src, dst = np.asarray(ei[0], np.int64), np.asarray(ei[1], np.int64)

    # multiplicity matrices M[g, s, d] (+ self loops)
    g_of = src // NPG
    sl = src - g_of * NPG
    dl = dst - (dst // NPG) * NPG
    flat = g_of * (NPG * NPG) + sl * NPG + dl
    Mall = np.bincount(flat, minlength=B * NPG * NPG).astype(np.float32).reshape(B, NPG, NPG)
    Mall[:, np.arange(NPG), np.arange(NPG)] += 1.0

    xg = x.reshape(B, NPG, F_IN)

    def mk_asad(a_s, a_d):
        a_s = np.asarray(a_s, np.float32)
        a_d = np.asarray(a_d, np.float32)
        out = np.zeros((HC, 12), np.float32)
        for h in range(H):
            out[h * C:(h + 1) * C, h] = a_s[h]
            out[h * C:(h + 1) * C, 6 + h] = a_d[h]
        return out

    w1 = np.asarray(inputs["w1"], np.float32)
    w2 = np.asarray(inputs["w2"], np.float32)
    was1 = w1 @ mk_asad(inputs["as1"], inputs["ad1"])   # [228, 12]
    was2 = w2 @ mk_asad(inputs["as2"], inputs["ad2"])   # [384, 12]

    # kernel folds assume zero biases / unit mean-scale (true for this model)
    for nm in ("b1", "b2", "gn1_b", "gn2_b"):
        assert np.abs(np.asarray(inputs[nm])).max() == 0.0, f"{nm} nonzero"
    for nm in ("gn1_ms", "gn2_ms"):
        assert np.abs(np.asarray(inputs[nm]) - 1.0).max() == 0.0, f"{nm} != 1"

    bn_w = np.asarray(inputs["bn_w"], np.float64)
    bn_b = np.asarray(inputs["bn_b"], np.float64)
    bn_rm = np.asarray(inputs["bn_rm"], np.float64)
    bn_rv = np.asarray(inputs["bn_rv"], np.float64)
    bn_sc = bn_w / np.sqrt(bn_rv + 1e-5)
    bn_sh = bn_b - bn_rm * bn_sc
    head64 = np.stack([np.asarray(inputs["lin1_b"], np.float64),
                       bn_sc, bn_sh, np.zeros((C,))], axis=1).astype(np.float32)

    # lin1 reorder: rows j=(n*384 + ck*128 + p) -> chunks (ck, n) of k=128
    lwt = np.asarray(inputs["lin1_w"], np.float32).reshape(NPG, 3, 128, C)
    lin1s = np.ascontiguousarray(lwt.transpose(2, 1, 0, 3)).reshape(128, NJ1 * C) \
        .astype(ml_dtypes.bfloat16)

    def cm(a):
        """[g, n, f] -> [114 (f-part), (fc, g, n)] bf16 channel-major."""
        gg, nn, ff = a.shape
        nkc = ff // NH
        t = a.transpose(2, 0, 1).reshape(nkc, NH, gg, nn).transpose(1, 0, 2, 3)
        return np.ascontiguousarray(t).reshape(NH, nkc * gg * nn).astype(ml_dtypes.bfloat16)

    gnc1 = np.zeros((128, 4), np.float32)
    gnc2 = np.zeros((128, 4), np.float32)
    gnc1[:, 0:3] = np.asarray(inputs["gn1_w"], np.float32).reshape(3, 128).T
    gnc2[:, 0:3] = np.asarray(inputs["gn2_w"], np.float32).reshape(3, 128).T

    dpati = np.zeros((7, 6 * NG), np.float32)
    for j in range(7 - 1):
        dpati[j, j * NG:(j + 1) * NG] = 1.0
    shared = dict(
        dpati=dpati.astype(ml_dtypes.bfloat16),
        onesi=np.ones((1, NG), ml_dtypes.bfloat16),
        w1s=np.ascontiguousarray(
            w1.reshape(2, NH, HC).transpose(1, 0, 2)).reshape(NH, 2 * HC)
            .astype(ml_dtypes.bfloat16),
        w2s=np.ascontiguousarray(
            w2.reshape(3, 128, HC).transpose(1, 0, 2)).reshape(128, 3 * HC)
            .astype(ml_dtypes.bfloat16),
        was1=np.ascontiguousarray(
            was1.reshape(2, NH, 12).transpose(1, 0, 2)).reshape(NH, 24)
            .astype(ml_dtypes.bfloat16),
        was2=np.ascontiguousarray(
            was2.reshape(3, 128, 12).transpose(1, 0, 2)).reshape(128, 36)
            .astype(ml_dtypes.bfloat16),
        gncol=gnc1, gncol2=gnc2,
        lin1s=lin1s, head64=head64,
        lin2w=np.asarray(inputs["lin2_w"], np.float32),
        lin2b=np.asarray(inputs["lin2_b"], np.float32).reshape(NCLS, 1),
    )

    in_maps = []
    for core in range(NCORES):
        gs = slice(core * GPC, (core + 1) * GPC)
        m = dict(shared)
        m["xb"] = cm(xg[gs])                           # [114, (fc, g, n)]
        m["mm"] = cm(Mall[gs].transpose(0, 2, 1))      # [114 (s), (sc, g, d)]
        in_maps.append(m)
    return in_maps


_cached_nc = None


def kernel(**inputs):
    global _cached_nc
    in_maps = _host_prep(inputs)
    if _cached_nc is None:
        _cached_nc = _build_program()
    nc = _cached_nc
    res = bass_utils.run_bass_kernel_spmd(nc, in_maps, core_ids=list(range(NCORES)))
    _last_results["exec_time_ns"] = res.exec_time_ns
    _last_results["res"] = res
    out = np.zeros((B, NCLS), np.float32)
    for core in range(NCORES):
        o = res.results[core]["out"]          # [2, 6]
        out[core * GPC:(core + 1) * GPC, :] = o.T
    return out



# revision 40
# speedup vs baseline: 1.0659x; 1.0091x over previous
"""GAT (2-layer, 6-head) + GraphNorm + readout MLP on 8 Trainium2 cores.

Sharding: graph-level data parallelism. 48 fixed-size graphs (228 nodes,
edges never cross graphs) -> 6 graphs per core. Weights replicated.

v2 redesign vs the per-graph baseline:
  - All 6 graphs batched per stage; channel-major [c, (g, n)] primary layout.
  - Dense attention scores z[s,d] built with GpSimd partition-broadcast of
    the a2 rows + one wide DVE add per graph ([114, 2*6*228] bf16 tiles,
    both source-halves per instruction), lrelu on DVE, exp on Scalar
    (single activation table: exp/ln/relu/copy), multiplicity mask on GpSimd.
  - Attention logits a1/a2 computed straight from the layer input with
    host-folded was = W @ [as|ad].
  - Aggregation: dest-partition matmuls with a fused ones-column so the
    softmax denominator falls out of the same PSUM tile.
  - lin1 readout: weights host-reordered to (ck, n, p) chunks of k=128 so
    the GEMV consumes the channel-major layer-2 output directly; weights
    streamed in 4 double-buffered DMA chunks overlapping the layer phase.

kernel(**inputs) -> np.ndarray [48, 2] float32.
"""
import sys
sys.path.insert(0, '/opt/trn_rl_repo')

import numpy as np

import concourse.bass as bass
import concourse.bacc as bacc
import concourse.mybir as mybir
import concourse.tile as tile
from concourse import masks
from concourse import bass_utils

F32 = mybir.dt.float32
BF16 = mybir.dt.bfloat16
Alu = mybir.AluOpType
Act = mybir.ActivationFunctionType

H, C = 6, 64
HC = 384
NPG = 228          # nodes per graph
B = 48             # graphs
GPC = 6            # graphs per core
NCORES = 8
F_IN = 228
NH = 114           # node half-chunk
NCLS = 2
NG = GPC * NPG     # 1368 node-columns per core
NJ1 = 3 * NPG      # 684 lin1 k-chunks of 128
NLCH = 6           # lin1 weight stream chunks (even JPC so FWL pairs don't split)
JPC = NJ1 // NLCH  # 114 chunks per stream piece

_last_results = {"exec_time_ns": None}


def _ensure_axon_hooks():
    """Make BASS_TRACE-driven NTFF profiling under axon degrade gracefully."""
    try:
        import antenv.axon_hooks  # noqa: F401
        return
    except ImportError:
        pass
    import types
    try:
        import antenv
    except ImportError:
        return
    mod = types.ModuleType("antenv.axon_hooks")
    holder = {"hook": None}
    mod.set_axon_ntff_profile_hook = lambda h: holder.__setitem__("hook", h)
    mod.get_axon_ntff_profile_hook = lambda: holder["hook"]
    sys.modules["antenv.axon_hooks"] = mod
    antenv.axon_hooks = mod
    try:
        from trn_agent_boot.trn_boot import _ntff_profile_via_ctypes
        hook = _ntff_profile_via_ctypes('/opt/axon/libaxon_pjrt.so')
        if hook is not None:
            mod.set_axon_ntff_profile_hook(hook)
    except Exception:
        pass
    _orig_upload = bass_utils.upload_artifacts

    def _safe_upload(tmpdir):
        try:
            return _orig_upload(tmpdir)
        except Exception:
            return "local://" + str(tmpdir)

    bass_utils.upload_artifacts = _safe_upload


_ensure_axon_hooks()


def _build_program():
    nc = bacc.Bacc("TRN2", target_bir_lowering=False, debug=False)

    dt_in = {}

    def din(name, shape, dtype=F32):
        t = nc.dram_tensor(name, shape, dtype, kind="ExternalInput")
        dt_in[name] = t
        return t

    din("xb", [NH, 2 * NG], BF16)            # x chan-major [p, (fc, g, n)]
    din("mm", [NH, 2 * NG], BF16)            # multiplicity+I [p, (sc, g, d)]
    din("w1s", [NH, 2 * HC], BF16)           # W1 [p, (fc, 384)]
    din("w2s", [128, 3 * HC], BF16)          # W2 [p, (kc, 384)]
    din("was1", [NH, 2 * 12], BF16)          # W1@[as|ad] [p, (fc, 12)]
    din("was2", [128, 3 * 12], BF16)
    din("gncol", [128, 4], F32)              # graphnorm gamma, col ck
    din("gncol2", [128, 4], F32)
    din("lin1s", [128, NJ1 * C], BF16)       # lin1_w reordered (p, (ck, n, 64))
    din("dpati", [7, 6 * NG], BF16)          # block-diag head-selector pattern
    din("onesi", [1, NG], BF16)
    din("head64", [C, 4], F32)               # cols: lin1_b, bn_scale, bn_shift
    din("lin2w", [C, NCLS], F32)
    din("lin2b", [NCLS, 1], F32)

    out_d = nc.dram_tensor("out", [NCLS, GPC], F32, kind="ExternalOutput")

    with tile.TileContext(nc) as tc:
        _emit(tc, dt_in, out_d)

    nc.finalize()
    return nc


def _emit(tc, din, out_d):
    nc = tc.nc

    cst = tc.alloc_tile_pool(name="cst", bufs=1)
    lw = tc.alloc_tile_pool(name="lw", bufs=6)
    hp = tc.alloc_tile_pool(name="hp", bufs=1)
    att = tc.alloc_tile_pool(name="att", bufs=1)
    scp = tc.alloc_tile_pool(name="scp", bufs=2)
    agw = tc.alloc_tile_pool(name="agw", bufs=2)
    xo = tc.alloc_tile_pool(name="xo", bufs=1)
    wk = tc.alloc_tile_pool(name="wk", bufs=2)
    psH = tc.alloc_tile_pool(name="psH", bufs=1, space="PSUM")
    psS = tc.alloc_tile_pool(name="psS", bufs=1, space="PSUM")
    psZ = tc.alloc_tile_pool(name="psZ", bufs=2, space="PSUM")
    psN = tc.alloc_tile_pool(name="psN", bufs=2, space="PSUM")
    psT = tc.alloc_tile_pool(name="psT", bufs=1, space="PSUM")
    psY = tc.alloc_tile_pool(name="psY", bufs=1, space="PSUM")

    # ---- inputs: latency-critical tensors lead BOTH queues; the dpat
    # patterns and late constants follow; the big lin1 stream comes last ----
    identb = cst.tile([128, 128], BF16)
    masks.make_identity(nc, identb[:])

    dpats, a1os = [], []
    for lay in range(2):
        dp = cst.tile([7, 6 * NG], BF16, name=f"dpat{lay}")
        nc.sync.dma_start(dp[:], din["dpati"].ap()[:, :])
        ao = cst.tile([7, NG], BF16, name=f"a1o{lay}")
        nc.sync.dma_start(ao[6:7, :], din["onesi"].ap()[0:1, :])
        dpats.append(dp)
        a1os.append(ao)

    xb = cst.tile([NH, 2 * NG], BF16)
    nc.sync.dma_start(xb[:, 0:NG], din["xb"].ap()[:, 0:NG])
    nc.scalar.dma_start(xb[:, NG:2 * NG], din["xb"].ap()[:, NG:2 * NG])
    w1s = cst.tile([NH, 2 * HC], BF16)
    nc.sync.dma_start(w1s[:], din["w1s"].ap()[:, :])
    was1 = cst.tile([NH, 2 * 12], BF16)
    nc.sync.dma_start(was1[:], din["was1"].ap()[:, :])
    mmt = cst.tile([NH, 2 * NG], BF16)
    nc.sync.dma_start(mmt[:, 0:NG], din["mm"].ap()[:, 0:NG])
    nc.scalar.dma_start(mmt[:, NG:2 * NG], din["mm"].ap()[:, NG:2 * NG])

    w2s = cst.tile([128, 3 * HC], BF16)
    nc.sync.dma_start(w2s[:], din["w2s"].ap()[:, :])
    was2 = cst.tile([128, 3 * 12], BF16)
    nc.sync.dma_start(was2[:], din["was2"].ap()[:, :])
    gncol = cst.tile([128, 4], F32)
    nc.sync.dma_start(gncol[:], din["gncol"].ap()[:, :])
    gncol2 = cst.tile([128, 4], F32)
    nc.sync.dma_start(gncol2[:], din["gncol2"].ap()[:, :])
    head64 = cst.tile([C, 4], F32)
    nc.sync.dma_start(head64[:], din["head64"].ap()[:, :])
    lin2w = cst.tile([C, NCLS], F32)
    nc.sync.dma_start(lin2w[:], din["lin2w"].ap()[:, :])
    lin2b = cst.tile([NCLS, 1], F32)
    nc.sync.dma_start(lin2b[:], din["lin2b"].ap()[:, :])

    # lin1 weight stream, split across BOTH hardware DMA queues: chunks
    # 0/2/4/5 ride the Activation queue from the start; chunks 1/3 ride the
    # sync queue in the idle window between the two layers' dpat relocations.
    lin1_t = [None] * NLCH

    def lin1_chunk(i, eng, gate=None):
        t = lw.tile([128, JPC * C], BF16, tag="lin1", name=f"lin1c{i}")
        if gate is not None:
            # WAW gate on the idle Scalar engine: holds the big stream back
            # until the critical input DMAs drain (shared DMA engines would
            # otherwise starve them)
            nc.scalar.copy(t[0:1, 0:1], gate)
        eng.dma_start(t[:], din["lin1s"].ap()[:, i * JPC * C:(i + 1) * JPC * C])
        lin1_t[i] = t

    def emit_scalar_chunks():
        for i in (0, 2, 4, 5):
            lin1_chunk(i, nc.scalar, gate=mmt[0:1, NG - 1:NG])

    def layer(xBs, wts, wast, gcol, lay, hook_g1=None):
        """One GAT layer + elu + graphnorm for all 6 graphs.

        xBs: list of nkc channel-major input tiles [p, (g, n)] bf16.
        wts: [p, (kc, 384)] bf16; wast: [p, (kc, 12)] bf16.
        Returns one tile [128, (ck, g, n)] bf16 channel-major.

        Scores use exp(lrelu(a1+a2)) = max(exp(a1)exp(a2), exp(.2a1)exp(.2a2)):
        each product is rank-1 per head, so the dense [s, (h,d)] score tile is
        built by a single k=6 matmul against a block-diagonal exp(a2) operand
        instead of partition-broadcasts + dense scalar activations."""
        nkc = len(xBs)
        dpat = dpats[lay]
        a1o = a1os[lay]

        # attention logits a1/a2 = was.T @ x as separate [6, nb] matmuls so both
        # land at partition base 0
        a2T = att.tile([6, NG], BF16, tag="a2T")
        for nb in range(3):
            cols = slice(nb * 456, (nb + 1) * 456)
            a1_ps = psS.tile([6, 456], F32, tag="aps")
            for kc in range(nkc):
                nc.tensor.matmul(a1_ps[:], wast[:, kc * 12:kc * 12 + 6],
                                 xBs[kc][:, cols],
                                 start=(kc == 0), stop=(kc == nkc - 1))
            nc.vector.tensor_copy(a1o[0:6, cols], a1_ps[:])
            a2_ps = psS.tile([6, 456], F32, tag="aps")
            for kc in range(nkc):
                nc.tensor.matmul(a2_ps[:], wast[:, kc * 12 + 6:kc * 12 + 12],
                                 xBs[kc][:, cols],
                                 start=(kc == 0), stop=(kc == nkc - 1))
            nc.vector.tensor_copy(a2T[:, cols], a2_ps[:])
        # relocate a2 rows into dpat row 6 (partition shift), per nb chunk so
        # early graphs' score operands are ready before the last graph finishes
        for nb in range(3):
            for h in range(6):
                nc.sync.dma_start(
                    dpat[6:7, h * NG + nb * 456: h * NG + (nb + 1) * 456],
                    a2T[h:h + 1, nb * 456:(nb + 1) * 456])

        # h node-major [114, (sc, g, h, 65)] bf16 directly from channel-major
        # input; 65th col = 1 (ones written once per layer, disjoint cols)
        hA65 = hp.tile([NH, 2 * GPC * 390], BF16, tag=f"hA65{lay}")
        nc.gpsimd.memset(
            hA65[:].rearrange("p (b h c) -> p b h c", h=6, c=65)[:, :, :, 64:65], 1.0)
        for sc in range(2):
            for g in range(GPC):
                h_ps = psH.tile([NH, HC], F32, tag="hps")
                col0 = g * NPG + sc * NH
                for kc in range(nkc):
                    nc.tensor.matmul(h_ps[:], xBs[kc][:, col0:col0 + NH],
                                     wts[:, kc * HC:(kc + 1) * HC],
                                     start=(kc == 0), stop=(kc == nkc - 1))
                dst = hA65[:, (sc * GPC + g) * 390:(sc * GPC + g + 1) * 390] \
                    .rearrange("p (h c) -> p h c", c=65)
                nc.scalar.copy(dst[:, :, 0:64],
                               h_ps[:].rearrange("p (h c) -> p h c", h=6))

        # ---- attention + aggregation per graph ----
        xout = xo.tile([128, 3 * NG], BF16, tag=f"xn{lay}", name=f"xn{lay}")
        xeA = xo.tile([128, 3 * NG], BF16, tag="xeA")
        meanA = wk.tile([128, 3 * GPC], F32, tag="meanA")
        vepsA = wk.tile([128, 3 * GPC], F32, tag="vepsA")
        for g in range(GPC):
            if g == 1 and hook_g1 is not None:
                hook_g1()
            # dense scores zs[sc][s, (h, dpad 256)] = exp(lrelu(a1+a2)) * mult,
            # logits built in head-pair chunks by k=7 matmuls. Each head's d
            # dim is padded to 256 zero-backed columns so the aggregation
            # stationaries are full 128-column (FWL-eligible) loads.
            dv = dpat[:].rearrange("k (h g d) -> k h g d", g=GPC, d=NPG)
            zs = []
            for sc in range(2):
                zt = scp.tile([NH, 6 * 256], BF16, tag=f"z{sc}")
                ztv = zt[:].rearrange("p (h d) -> p h d", d=256)
                if lay == 0 and g < 2:
                    nc.vector.memset(ztv[:, :, NPG:256], 0.0)
                scol = g * NPG + sc * NH
                for hq in range(3):
                    e_ps = psZ.tile([NH, 2 * NPG], F32, tag="zz")
                    nc.tensor.matmul(e_ps[:], a1o[:, scol:scol + NH],
                                     dv[:, hq * 2:(hq + 1) * 2, g, :],
                                     start=True, stop=True)
                    zq = ztv[:, hq * 2:(hq + 1) * 2, 0:NPG]
                    eps2 = e_ps[:].rearrange("p (h d) -> p h d", h=2)
                    nc.scalar.activation(zq, eps2, Act.Prelu, alpha=0.2)
                    nc.scalar.activation(zq, zq, Act.Exp)
                    for hh in range(2):
                        h = hq * 2 + hh
                        zh = ztv[:, h, 0:NPG]
                        nc.vector.tensor_tensor(
                            out=zh, in0=zh,
                            in1=mmt[:, (sc * GPC + g) * NPG:(sc * GPC + g + 1) * NPG],
                            op=Alu.mult)
                zs.append(zt)

            # aggregation: psum [d, (h, 65)] per dc chunk (d 0:128 | 128:228);
            # col 64 = denominator. dc alternates between the two psN banks so
            # PSUM drains overlap; stationaries are 128-col FWL loads.
            dlen = (128, 100)
            x2p = agw.tile([128, 2 * HC], BF16, tag="x2p")
            n_ps = [psN.tile([128, 390], F32, tag="nps", name=f"nps{dd}") for dd in range(2)]
            for h in range(6):
                for dc in range(2):
                    for sc in range(2):
                        nc.tensor.matmul(
                            n_ps[dc][:, h * 65:(h + 1) * 65],
                            zs[sc][:, h * 256 + dc * 128: h * 256 + (dc + 1) * 128],
                            hA65[:, (sc * GPC + g) * 390 + h * 65:(sc * GPC + g) * 390 + (h + 1) * 65],
                            start=(sc == 0), stop=(sc == 1))
            for dc in range(2):
                dl = dlen[dc]
                rec = agw.tile([128, 6], F32, tag="rec")
                nc.vector.reciprocal(
                    rec[0:dl, :],
                    n_ps[dc][0:dl].rearrange("p (h c) -> p h c", c=65)[:, :, 64:65]
                    .rearrange("p h c -> p (h c)"))
                nc.vector.tensor_tensor(
                    out=x2p[0:dl, dc * HC:(dc + 1) * HC].rearrange("p (h c) -> p h c", h=6),
                    in0=n_ps[dc][0:dl].rearrange("p (h c) -> p h c", c=65)[:, :, 0:64],
                    in1=rec[0:dl].rearrange("p (h c) -> p h c", c=1).broadcast_to((dl, 6, 64)),
                    op=Alu.mult)
            # transpose this graph's columns to channel-major right away
            xg = wk.tile([128, 3 * NPG], BF16, tag="xg")
            for ck in range(3):
                tp = psT.tile([128, 2 * NH], BF16, tag="tp")
                nc.tensor.transpose(
                    tp[:, 0:128],
                    x2p[:, ck * 128:(ck + 1) * 128],
                    identb[:])
                nc.tensor.transpose(
                    tp[:, 128:228],
                    x2p[0:100, HC + ck * 128: HC + (ck + 1) * 128],
                    identb[0:100, 0:100])
                nc.scalar.copy(xg[:, ck * NPG:(ck + 1) * NPG], tp[:, 0:NPG])
            # elu over the whole graph at once, into the layer xe arena
            m = wk.tile([128, 3 * NPG], BF16, tag="m")
            nc.vector.tensor_scalar_min(m[:], xg[:], 0.0)
            nc.scalar.activation(m[:], m[:], Act.Exp)
            xe = xeA[:, g * 3 * NPG:(g + 1) * 3 * NPG]
            nc.vector.scalar_tensor_tensor(xe, m[:], -1.0, xg[:],
                                           op0=Alu.add, op1=Alu.max)
            # per-graph mean/var (DVE only; Ln/Exp batched to avoid activation
            # table swaps between exp and ln sets)
            xe3 = xe.rearrange("p (c n) -> p c n", c=3)
            s13 = wk.tile([128, 3], F32, tag="s13")
            nc.vector.tensor_reduce(s13[:], xe3, axis=mybir.AxisListType.X, op=Alu.add)
            sq = wk.tile([128, 3 * NPG], BF16, tag="sq")
            nc.vector.tensor_tensor(out=sq[:], in0=xe, in1=xe, op=Alu.mult)
            s23 = wk.tile([128, 3], F32, tag="s23")
            nc.vector.tensor_reduce(s23[:], sq[:].rearrange("p (c n) -> p c n", c=3),
                                    axis=mybir.AxisListType.X, op=Alu.add)
            mean3 = meanA[:, g * 3:(g + 1) * 3]
            nc.vector.tensor_scalar_mul(mean3, s13[:], 1.0 / NPG)
            msq3 = wk.tile([128, 3], F32, tag="msq3")
            nc.vector.tensor_tensor(out=msq3[:], in0=mean3, in1=mean3, op=Alu.mult)
            veps3 = vepsA[:, g * 3:(g + 1) * 3]
            nc.vector.scalar_tensor_tensor(veps3, s23[:], 1.0 / NPG, msq3[:],
                                           op0=Alu.mult, op1=Alu.subtract)
            nc.vector.tensor_scalar_add(veps3, veps3, 1e-5)

            def finish(gg):
                # out = xe * gisd - tcol   (gamma folded; beta==0)
                gisd3 = wk.tile([128, 3], F32, tag="gisd3")
                nc.vector.tensor_tensor(out=gisd3[:], in0=vepsA[:, gg * 3:(gg + 1) * 3],
                                        in1=gcol[:, 0:3], op=Alu.mult)
                tcol3 = wk.tile([128, 3], F32, tag="tcol3")
                nc.vector.tensor_tensor(out=tcol3[:], in0=meanA[:, gg * 3:(gg + 1) * 3],
                                        in1=gisd3[:], op=Alu.mult)
                for ck in range(3):
                    oc = xout[:, ck * NG + gg * NPG: ck * NG + (gg + 1) * NPG]
                    nc.vector.scalar_tensor_tensor(
                        oc, xeA[:, gg * 3 * NPG + ck * NPG: gg * 3 * NPG + (ck + 1) * NPG],
                        gisd3[:, ck:ck + 1],
                        tcol3[:, ck:ck + 1].broadcast_to((128, NPG)),
                        op0=Alu.mult, op1=Alu.subtract)

            if g == GPC - 2 or g == GPC - 1:
                # one Ln/Exp for graphs [0..4] at g==4, then [5] at g==5:
                # overwrite veps in place with 1/sqrt(veps)
                lo = 0 if g == GPC - 2 else (GPC - 1) * 3
                hi = (GPC - 1) * 3 if g == GPC - 2 else GPC * 3
                nc.scalar.activation(vepsA[:, lo:hi], vepsA[:, lo:hi], Act.Ln)
                nc.scalar.activation(vepsA[:, lo:hi], vepsA[:, lo:hi], Act.Exp,
                                     scale=-0.5)
                for gg in range(lo // 3, hi // 3):
                    finish(gg)
        return xout

    x2 = layer([xb[:, 0:NG], xb[:, NG:2 * NG]], w1s, was1, gncol, 0,
               hook_g1=emit_scalar_chunks)
    x2v = [x2[:, ck * NG:(ck + 1) * NG] for ck in range(3)]
    lin1_chunk(1, nc.sync)
    lin1_chunk(3, nc.sync)
    x3t = layer(x2v, w2s, was2, gncol2, 1)
    x3 = [x3t[:, ck * NG:(ck + 1) * NG] for ck in range(3)]

    # ---- lin1 GEMV: 684 k=128 chunks processed in PAIRS. Each pair loads a
    # full [128, 128] stationary tile (two adjacent nodes' weight chunks side
    # by side -> FWL-eligible) against a [128, 2, 6] moving slice. Diagonal
    # blocks of the [128, 12] psum hold the real partials; off-diagonal blocks
    # accumulate ignored cross terms. ----
    y_ps = psY.tile([128, 2 * GPC], F32, tag="y")
    for i in range(NLCH):
        lt = lin1_t[i]
        for jj in range(0, JPC, 2):
            jc = i * JPC + jj
            ck, n = jc // NPG, jc % NPG
            x3r = x3[ck].rearrange("p (g n) -> p n g", g=GPC)
            nc.tensor.matmul(y_ps[:], lt[:, jj * C:(jj + 2) * C],
                             x3r[:, n:n + 2, :],
                             start=(jc == 0), stop=(jc == NJ1 - 2))

    # fold: y = y_ps[0:64, 0:6] + y_ps[64:128, 6:12] (partition shift via DMA)
    yhi = wk.tile([128, GPC], F32, tag="yhi")
    nc.scalar.copy(yhi[64:128, :], y_ps[64:128, GPC:2 * GPC])
    ylo = wk.tile([C, GPC], F32, tag="ylo")
    nc.sync.dma_start(ylo[:], yhi[64:128, :])

    # ---- head: +b, elu, bn, lin2 ----
    yb = wk.tile([C, GPC], F32, tag="yb")
    nc.vector.scalar_tensor_tensor(yb[:], y_ps[0:C, 0:GPC], head64[:, 0:1],
                                   ylo[:], op0=Alu.add, op1=Alu.add)
    m2 = wk.tile([C, GPC], F32, tag="m2")
    nc.vector.tensor_scalar_min(m2[:], yb[:], 0.0)
    e2 = wk.tile([C, GPC], F32, tag="e2")
    nc.scalar.activation(e2[:], m2[:], Act.Exp)
    ye = wk.tile([C, GPC], F32, tag="ye")
    nc.vector.scalar_tensor_tensor(ye[:], e2[:], -1.0, yb[:], op0=Alu.add, op1=Alu.max)
    yn = wk.tile([C, GPC], F32, tag="yn")
    nc.vector.scalar_tensor_tensor(yn[:], ye[:], head64[:, 1:2],
                                   head64[:, 2:3].broadcast_to((C, GPC)),
                                   op0=Alu.mult, op1=Alu.add)
    o_ps = psY.tile([128, 2 * GPC], F32, tag="y")
    nc.tensor.matmul(o_ps[0:NCLS, 0:GPC], lin2w[:], yn[:], start=True, stop=True)
    ob = wk.tile([NCLS, GPC], F32, tag="ob")
    nc.vector.tensor_scalar_add(ob[:], o_ps[0:NCLS, 0:GPC], lin2b[:])
    nc.sync.dma_start(out_d.ap()[:, :], ob[:])

    for p in (psY, psT, psN, psZ, psS, psH, wk, xo, agw, scp, att, hp, lw, cst):
        p.release()


def _host_prep(inputs):
    """Build per-core input maps (sharding / relayout / dtype prep)."""
    import ml_dtypes
    x = np.asarray(inputs["x"], np.float32)
    ei = np.asarray(inputs["edge_index"])
    src, dst = np.asarray(ei[0], np.int64), np.asarray(ei[1], np.int64)

    # multiplicity matrices M[g, s, d] (+ self loops)
    g_of = src // NPG
    sl = src - g_of * NPG
    dl = dst - (dst // NPG) * NPG
    flat = g_of * (NPG * NPG) + sl * NPG + dl
    Mall = np.bincount(flat, minlength=B * NPG * NPG).astype(np.float32).reshape(B, NPG, NPG)
    Mall[:, np.arange(NPG), np.arange(NPG)] += 1.0

    xg = x.reshape(B, NPG, F_IN)

    def mk_asad(a_s, a_d):
        a_s = np.asarray(a_s, np.float32)
        a_d = np.asarray(a_d, np.float32)
        out = np.zeros((HC, 12), np.float32)
        for h in range(H):
            out[h * C:(h + 1) * C, h] = a_s[h]
            out[h * C:(h + 1) * C, 6 + h] = a_d[h]
        return out

    w1 = np.asarray(inputs["w1"], np.float32)
    w2 = np.asarray(inputs["w2"], np.float32)
    was1 = w1 @ mk_asad(inputs["as1"], inputs["ad1"])   # [228, 12]
    was2 = w2 @ mk_asad(inputs["as2"], inputs["ad2"])   # [384, 12]

    # kernel folds assume zero biases / unit mean-scale (true for this model)
    for nm in ("b1", "b2", "gn1_b", "gn2_b"):
        assert np.abs(np.asarray(inputs[nm])).max() == 0.0, f"{nm} nonzero"
    for nm in ("gn1_ms", "gn2_ms"):
        assert np.abs(np.asarray(inputs[nm]) - 1.0).max() == 0.0, f"{nm} != 1"

    bn_w = np.asarray(inputs["bn_w"], np.float64)
    bn_b = np.asarray(inputs["bn_b"], np.float64)
    bn_rm = np.asarray(inputs["bn_rm"], np.float64)
    bn_rv = np.asarray(inputs["bn_rv"], np.float64)
    bn_sc = bn_w / np.sqrt(bn_rv + 1e-5)
    bn_sh = bn_b - bn_rm * bn_sc
    head64 = np.stack([np.asarray(inputs["lin1_b"], np.float64),
                       bn_sc, bn_sh, np.zeros((C,))], axis=1).astype(np.float32)

    # lin1 reorder: rows j=(n*384 + ck*128 + p) -> chunks (ck, n) of k=128
    lwt = np.asarray(inputs["lin1_w"], np.float32).reshape(NPG, 3, 128, C)
    lin1s = np.ascontiguousarray(lwt.transpose(2, 1, 0, 3)).reshape(128, NJ1 * C) \
        .astype(ml_dtypes.bfloat16)

    def cm(a):
        """[g, n, f] -> [114 (f-part), (fc, g, n)] bf16 channel-major."""
        gg, nn, ff = a.shape
        nkc = ff // NH
        t = a.transpose(2, 0, 1).reshape(nkc, NH, gg, nn).transpose(1, 0, 2, 3)
        return np.ascontiguousarray(t).reshape(NH, nkc * gg * nn).astype(ml_dtypes.bfloat16)

    gnc1 = np.zeros((128, 4), np.float32)
    gnc2 = np.zeros((128, 4), np.float32)
    gnc1[:, 0:3] = np.asarray(inputs["gn1_w"], np.float32).reshape(3, 128).T
    gnc2[:, 0:3] = np.asarray(inputs["gn2_w"], np.float32).reshape(3, 128).T

    dpati = np.zeros((7, 6 * NG), np.float32)
    for j in range(7 - 1):
        dpati[j, j * NG:(j + 1) * NG] = 1.0
    shared = dict(
        dpati=dpati.astype(ml_dtypes.bfloat16),
        onesi=np.ones((1, NG), ml_dtypes.bfloat16),
        w1s=np.ascontiguousarray(
            w1.reshape(2, NH, HC).transpose(1, 0, 2)).reshape(NH, 2 * HC)
            .astype(ml_dtypes.bfloat16),
        w2s=np.ascontiguousarray(
            w2.reshape(3, 128, HC).transpose(1, 0, 2)).reshape(128, 3 * HC)
            .astype(ml_dtypes.bfloat16),
        was1=np.ascontiguousarray(
            was1.reshape(2, NH, 12).transpose(1, 0, 2)).reshape(NH, 24)
            .astype(ml_dtypes.bfloat16),
        was2=np.ascontiguousarray(
            was2.reshape(3, 128, 12).transpose(1, 0, 2)).reshape(128, 36)
            .astype(ml_dtypes.bfloat16),
        gncol=gnc1, gncol2=gnc2,
        lin1s=lin1s, head64=head64,
        lin2w=np.asarray(inputs["lin2_w"], np.float32),
        lin2b=np.asarray(inputs["lin2_b"], np.float32).reshape(NCLS, 1),
    )

    in_maps = []
    for core in range(NCORES):
        gs = slice(core * GPC, (core + 1) * GPC)
        m = dict(shared)
        m["xb"] = cm(xg[gs])                           # [114, (fc, g, n)]
        m["mm"] = cm(Mall[gs].transpose(0, 2, 1))      # [114 (s), (sc, g, d)]
        in_maps.append(m)
    return in_maps


_cached_nc = None


def kernel(**inputs):
    global _cached_nc
    in_maps = _host_prep(inputs)
    if _cached_nc is None:
        _cached_nc = _build_program()
    nc = _cached_nc
    res = bass_utils.run_bass_kernel_spmd(nc, in_maps, core_ids=list(range(NCORES)))
    _last_results["exec_time_ns"] = res.exec_time_ns
    _last_results["res"] = res
    out = np.zeros((B, NCLS), np.float32)
    for core in range(NCORES):
        o = res.results[core]["out"]          # [2, 6]
        out[core * GPC:(core + 1) * GPC, :] = o.T
    return out



# revision 41
# speedup vs baseline: 1.0686x; 1.0025x over previous
"""GAT (2-layer, 6-head) + GraphNorm + readout MLP on 8 Trainium2 cores.

Sharding: graph-level data parallelism. 48 fixed-size graphs (228 nodes,
edges never cross graphs) -> 6 graphs per core. Weights replicated.

v2 redesign vs the per-graph baseline:
  - All 6 graphs batched per stage; channel-major [c, (g, n)] primary layout.
  - Dense attention scores z[s,d] built with GpSimd partition-broadcast of
    the a2 rows + one wide DVE add per graph ([114, 2*6*228] bf16 tiles,
    both source-halves per instruction), lrelu on DVE, exp on Scalar
    (single activation table: exp/ln/relu/copy), multiplicity mask on GpSimd.
  - Attention logits a1/a2 computed straight from the layer input with
    host-folded was = W @ [as|ad].
  - Aggregation: dest-partition matmuls with a fused ones-column so the
    softmax denominator falls out of the same PSUM tile.
  - lin1 readout: weights host-reordered to (ck, n, p) chunks of k=128 so
    the GEMV consumes the channel-major layer-2 output directly; weights
    streamed in 4 double-buffered DMA chunks overlapping the layer phase.

kernel(**inputs) -> np.ndarray [48, 2] float32.
"""
import sys
sys.path.insert(0, '/opt/trn_rl_repo')

import numpy as np

import concourse.bass as bass
import concourse.bacc as bacc
import concourse.mybir as mybir
import concourse.tile as tile
from concourse import masks
from concourse import bass_utils

F32 = mybir.dt.float32
BF16 = mybir.dt.bfloat16
Alu = mybir.AluOpType
Act = mybir.ActivationFunctionType

H, C = 6, 64
HC = 384
NPG = 228          # nodes per graph
B = 48             # graphs
GPC = 6            # graphs per core
NCORES = 8
F_IN = 228
NH = 114           # node half-chunk
NCLS = 2
NG = GPC * NPG     # 1368 node-columns per core
NJ1 = 3 * NPG      # 684 lin1 k-chunks of 128
NLCH = 6           # lin1 weight stream chunks (even JPC so FWL pairs don't split)
JPC = NJ1 // NLCH  # 114 chunks per stream piece

_last_results = {"exec_time_ns": None}


def _ensure_axon_hooks():
    """Make BASS_TRACE-driven NTFF profiling under axon degrade gracefully."""
    try:
        import antenv.axon_hooks  # noqa: F401
        return
    except ImportError:
        pass
    import types
    try:
        import antenv
    except ImportError:
        return
    mod = types.ModuleType("antenv.axon_hooks")
    holder = {"hook": None}
    mod.set_axon_ntff_profile_hook = lambda h: holder.__setitem__("hook", h)
    mod.get_axon_ntff_profile_hook = lambda: holder["hook"]
    sys.modules["antenv.axon_hooks"] = mod
    antenv.axon_hooks = mod
    try:
        from trn_agent_boot.trn_boot import _ntff_profile_via_ctypes
        hook = _ntff_profile_via_ctypes('/opt/axon/libaxon_pjrt.so')
        if hook is not None:
            mod.set_axon_ntff_profile_hook(hook)
    except Exception:
        pass
    _orig_upload = bass_utils.upload_artifacts

    def _safe_upload(tmpdir):
        try:
            return _orig_upload(tmpdir)
        except Exception:
            return "local://" + str(tmpdir)

    bass_utils.upload_artifacts = _safe_upload


_ensure_axon_hooks()


def _build_program():
    nc = bacc.Bacc("TRN2", target_bir_lowering=False, debug=False)

    dt_in = {}

    def din(name, shape, dtype=F32):
        t = nc.dram_tensor(name, shape, dtype, kind="ExternalInput")
        dt_in[name] = t
        return t

    din("xb", [NH, 2 * NG], BF16)            # x chan-major [p, (fc, g, n)]
    din("mm", [NH, 2 * NG], BF16)            # multiplicity+I [p, (sc, g, d)]
    din("w1s", [NH, 2 * HC], BF16)           # W1 [p, (fc, 384)]
    din("w2s", [128, 3 * HC], BF16)          # W2 [p, (kc, 384)]
    din("was1", [NH, 2 * 12], BF16)          # W1@[as|ad] [p, (fc, 12)]
    din("was2", [128, 3 * 12], BF16)
    din("gncol", [128, 4], F32)              # graphnorm gamma, col ck
    din("gncol2", [128, 4], F32)
    din("lin1s", [128, NJ1 * C], BF16)       # lin1_w reordered (p, (ck, n, 64))
    din("dpati", [7, 6 * NG], BF16)          # block-diag head-selector pattern
    din("onesi", [1, NG], BF16)
    din("head64", [C, 4], F32)               # cols: lin1_b, bn_scale, bn_shift
    din("lin2w", [C, NCLS], F32)
    din("lin2b", [NCLS, 1], F32)

    out_d = nc.dram_tensor("out", [NCLS, GPC], F32, kind="ExternalOutput")

    with tile.TileContext(nc) as tc:
        _emit(tc, dt_in, out_d)

    nc.finalize()
    return nc


def _emit(tc, din, out_d):
    nc = tc.nc

    cst = tc.alloc_tile_pool(name="cst", bufs=1)
    lw = tc.alloc_tile_pool(name="lw", bufs=6)
    hp = tc.alloc_tile_pool(name="hp", bufs=1)
    att = tc.alloc_tile_pool(name="att", bufs=1)
    scp = tc.alloc_tile_pool(name="scp", bufs=2)
    agw = tc.alloc_tile_pool(name="agw", bufs=2)
    xo = tc.alloc_tile_pool(name="xo", bufs=1)
    wk = tc.alloc_tile_pool(name="wk", bufs=2)
    psH = tc.alloc_tile_pool(name="psH", bufs=1, space="PSUM")
    psS = tc.alloc_tile_pool(name="psS", bufs=1, space="PSUM")
    psZ = tc.alloc_tile_pool(name="psZ", bufs=2, space="PSUM")
    psN = tc.alloc_tile_pool(name="psN", bufs=2, space="PSUM")
    psT = tc.alloc_tile_pool(name="psT", bufs=1, space="PSUM")
    psY = tc.alloc_tile_pool(name="psY", bufs=1, space="PSUM")

    # ---- inputs: latency-critical tensors lead BOTH queues; the dpat
    # patterns and late constants follow; the big lin1 stream comes last ----
    identb = cst.tile([128, 128], BF16)
    masks.make_identity(nc, identb[:])

    dpats, a1os = [], []
    for lay in range(2):
        dp = cst.tile([7, 6 * NG], BF16, name=f"dpat{lay}")
        nc.sync.dma_start(dp[:], din["dpati"].ap()[:, :])
        ao = cst.tile([7, NG], BF16, name=f"a1o{lay}")
        nc.sync.dma_start(ao[6:7, :], din["onesi"].ap()[0:1, :])
        dpats.append(dp)
        a1os.append(ao)

    xb = cst.tile([NH, 2 * NG], BF16)
    nc.sync.dma_start(xb[:, 0:456], din["xb"].ap()[:, 0:456])
    nc.scalar.dma_start(xb[:, NG:NG + 456], din["xb"].ap()[:, NG:NG + 456])
    nc.sync.dma_start(xb[:, 456:NG], din["xb"].ap()[:, 456:NG])
    nc.scalar.dma_start(xb[:, NG + 456:2 * NG], din["xb"].ap()[:, NG + 456:2 * NG])
    w1s = cst.tile([NH, 2 * HC], BF16)
    nc.sync.dma_start(w1s[:], din["w1s"].ap()[:, :])
    was1 = cst.tile([NH, 2 * 12], BF16)
    nc.sync.dma_start(was1[:], din["was1"].ap()[:, :])
    mmt = cst.tile([NH, 2 * NG], BF16)
    nc.sync.dma_start(mmt[:, 0:NG], din["mm"].ap()[:, 0:NG])
    nc.scalar.dma_start(mmt[:, NG:2 * NG], din["mm"].ap()[:, NG:2 * NG])

    w2s = cst.tile([128, 3 * HC], BF16)
    nc.sync.dma_start(w2s[:], din["w2s"].ap()[:, :])
    was2 = cst.tile([128, 3 * 12], BF16)
    nc.sync.dma_start(was2[:], din["was2"].ap()[:, :])
    gncol = cst.tile([128, 4], F32)
    nc.sync.dma_start(gncol[:], din["gncol"].ap()[:, :])
    gncol2 = cst.tile([128, 4], F32)
    nc.sync.dma_start(gncol2[:], din["gncol2"].ap()[:, :])
    head64 = cst.tile([C, 4], F32)
    nc.sync.dma_start(head64[:], din["head64"].ap()[:, :])
    lin2w = cst.tile([C, NCLS], F32)
    nc.sync.dma_start(lin2w[:], din["lin2w"].ap()[:, :])
    lin2b = cst.tile([NCLS, 1], F32)
    nc.sync.dma_start(lin2b[:], din["lin2b"].ap()[:, :])

    # lin1 weight stream, split across BOTH hardware DMA queues: chunks
    # 0/2/4/5 ride the Activation queue from the start; chunks 1/3 ride the
    # sync queue in the idle window between the two layers' dpat relocations.
    lin1_t = [None] * NLCH

    def lin1_chunk(i, eng, gate=None):
        t = lw.tile([128, JPC * C], BF16, tag="lin1", name=f"lin1c{i}")
        if gate is not None:
            # WAW gate on the idle Scalar engine: holds the big stream back
            # until the critical input DMAs drain (shared DMA engines would
            # otherwise starve them)
            nc.scalar.copy(t[0:1, 0:1], gate)
        eng.dma_start(t[:], din["lin1s"].ap()[:, i * JPC * C:(i + 1) * JPC * C])
        lin1_t[i] = t

    def emit_scalar_chunks():
        for i in (0, 2, 4, 5):
            lin1_chunk(i, nc.scalar, gate=mmt[0:1, NG - 1:NG])

    def layer(xBs, wts, wast, gcol, lay, hook_g1=None):
        """One GAT layer + elu + graphnorm for all 6 graphs.

        xBs: list of nkc channel-major input tiles [p, (g, n)] bf16.
        wts: [p, (kc, 384)] bf16; wast: [p, (kc, 12)] bf16.
        Returns one tile [128, (ck, g, n)] bf16 channel-major.

        Scores use exp(lrelu(a1+a2)) = max(exp(a1)exp(a2), exp(.2a1)exp(.2a2)):
        each product is rank-1 per head, so the dense [s, (h,d)] score tile is
        built by a single k=6 matmul against a block-diagonal exp(a2) operand
        instead of partition-broadcasts + dense scalar activations."""
        nkc = len(xBs)
        dpat = dpats[lay]
        a1o = a1os[lay]

        # attention logits a1/a2 = was.T @ x as separate [6, nb] matmuls so both
        # land at partition base 0
        a2T = att.tile([6, NG], BF16, tag="a2T")
        for nb in range(3):
            cols = slice(nb * 456, (nb + 1) * 456)
            a1_ps = psS.tile([6, 456], F32, tag="aps")
            for kc in range(nkc):
                nc.tensor.matmul(a1_ps[:], wast[:, kc * 12:kc * 12 + 6],
                                 xBs[kc][:, cols],
                                 start=(kc == 0), stop=(kc == nkc - 1))
            nc.vector.tensor_copy(a1o[0:6, cols], a1_ps[:])
            a2_ps = psS.tile([6, 456], F32, tag="aps")
            for kc in range(nkc):
                nc.tensor.matmul(a2_ps[:], wast[:, kc * 12 + 6:kc * 12 + 12],
                                 xBs[kc][:, cols],
                                 start=(kc == 0), stop=(kc == nkc - 1))
            nc.vector.tensor_copy(a2T[:, cols], a2_ps[:])
        # relocate a2 rows into dpat row 6 (partition shift), per nb chunk so
        # early graphs' score operands are ready before the last graph finishes
        for nb in range(3):
            for h in range(6):
                nc.sync.dma_start(
                    dpat[6:7, h * NG + nb * 456: h * NG + (nb + 1) * 456],
                    a2T[h:h + 1, nb * 456:(nb + 1) * 456])

        # h node-major [114, (sc, g, h, 65)] bf16 directly from channel-major
        # input; 65th col = 1 (ones written once per layer, disjoint cols)
        hA65 = hp.tile([NH, 2 * GPC * 390], BF16, tag=f"hA65{lay}")
        nc.gpsimd.memset(
            hA65[:].rearrange("p (b h c) -> p b h c", h=6, c=65)[:, :, :, 64:65], 1.0)
        for sc in range(2):
            for g in range(GPC):
                h_ps = psH.tile([NH, HC], F32, tag="hps")
                col0 = g * NPG + sc * NH
                for kc in range(nkc):
                    nc.tensor.matmul(h_ps[:], xBs[kc][:, col0:col0 + NH],
                                     wts[:, kc * HC:(kc + 1) * HC],
                                     start=(kc == 0), stop=(kc == nkc - 1))
                dst = hA65[:, (sc * GPC + g) * 390:(sc * GPC + g + 1) * 390] \
                    .rearrange("p (h c) -> p h c", c=65)
                nc.scalar.copy(dst[:, :, 0:64],
                               h_ps[:].rearrange("p (h c) -> p h c", h=6))

        # ---- attention + aggregation per graph ----
        xout = xo.tile([128, 3 * NG], BF16, tag=f"xn{lay}", name=f"xn{lay}")
        xeA = xo.tile([128, 3 * NG], BF16, tag="xeA")
        meanA = wk.tile([128, 3 * GPC], F32, tag="meanA")
        vepsA = wk.tile([128, 3 * GPC], F32, tag="vepsA")
        for g in range(GPC):
            if g == 1 and hook_g1 is not None:
                hook_g1()
            # dense scores zs[sc][s, (h, dpad 256)] = exp(lrelu(a1+a2)) * mult,
            # logits built in head-pair chunks by k=7 matmuls. Each head's d
            # dim is padded to 256 zero-backed columns so the aggregation
            # stationaries are full 128-column (FWL-eligible) loads.
            dv = dpat[:].rearrange("k (h g d) -> k h g d", g=GPC, d=NPG)
            zs = []
            for sc in range(2):
                zt = scp.tile([NH, 6 * 256], BF16, tag=f"z{sc}")
                ztv = zt[:].rearrange("p (h d) -> p h d", d=256)
                if lay == 0 and g < 2:
                    nc.vector.memset(ztv[:, :, NPG:256], 0.0)
                scol = g * NPG + sc * NH
                for hq in range(3):
                    e_ps = psZ.tile([NH, 2 * NPG], F32, tag="zz")
                    nc.tensor.matmul(e_ps[:], a1o[:, scol:scol + NH],
                                     dv[:, hq * 2:(hq + 1) * 2, g, :],
                                     start=True, stop=True)
                    zq = ztv[:, hq * 2:(hq + 1) * 2, 0:NPG]
                    eps2 = e_ps[:].rearrange("p (h d) -> p h d", h=2)
                    nc.scalar.activation(zq, eps2, Act.Prelu, alpha=0.2)
                    nc.scalar.activation(zq, zq, Act.Exp)
                    for hh in range(2):
                        h = hq * 2 + hh
                        zh = ztv[:, h, 0:NPG]
                        nc.vector.tensor_tensor(
                            out=zh, in0=zh,
                            in1=mmt[:, (sc * GPC + g) * NPG:(sc * GPC + g + 1) * NPG],
                            op=Alu.mult)
                zs.append(zt)

            # aggregation: psum [d, (h, 65)] per dc chunk (d 0:128 | 128:228);
            # col 64 = denominator. dc alternates between the two psN banks so
            # PSUM drains overlap; stationaries are 128-col FWL loads.
            dlen = (128, 100)
            x2p = agw.tile([128, 2 * HC], BF16, tag="x2p")
            n_ps = [psN.tile([128, 390], F32, tag="nps", name=f"nps{dd}") for dd in range(2)]
            for h in range(6):
                for dc in range(2):
                    for sc in range(2):
                        nc.tensor.matmul(
                            n_ps[dc][:, h * 65:(h + 1) * 65],
                            zs[sc][:, h * 256 + dc * 128: h * 256 + (dc + 1) * 128],
                            hA65[:, (sc * GPC + g) * 390 + h * 65:(sc * GPC + g) * 390 + (h + 1) * 65],
                            start=(sc == 0), stop=(sc == 1))
            for dc in range(2):
                dl = dlen[dc]
                rec = agw.tile([128, 6], F32, tag="rec")
                nc.vector.reciprocal(
                    rec[0:dl, :],
                    n_ps[dc][0:dl].rearrange("p (h c) -> p h c", c=65)[:, :, 64:65]
                    .rearrange("p h c -> p (h c)"))
                nc.vector.tensor_tensor(
                    out=x2p[0:dl, dc * HC:(dc + 1) * HC].rearrange("p (h c) -> p h c", h=6),
                    in0=n_ps[dc][0:dl].rearrange("p (h c) -> p h c", c=65)[:, :, 0:64],
                    in1=rec[0:dl].rearrange("p (h c) -> p h c", c=1).broadcast_to((dl, 6, 64)),
                    op=Alu.mult)
            # transpose this graph's columns to channel-major right away
            xg = wk.tile([128, 3 * NPG], BF16, tag="xg")
            for ck in range(3):
                tp = psT.tile([128, 2 * NH], BF16, tag="tp")
                nc.tensor.transpose(
                    tp[:, 0:128],
                    x2p[:, ck * 128:(ck + 1) * 128],
                    identb[:])
                nc.tensor.transpose(
                    tp[:, 128:228],
                    x2p[0:100, HC + ck * 128: HC + (ck + 1) * 128],
                    identb[0:100, 0:100])
                nc.scalar.copy(xg[:, ck * NPG:(ck + 1) * NPG], tp[:, 0:NPG])
            # elu over the whole graph at once, into the layer xe arena
            m = wk.tile([128, 3 * NPG], BF16, tag="m")
            nc.vector.tensor_scalar_min(m[:], xg[:], 0.0)
            nc.scalar.activation(m[:], m[:], Act.Exp)
            xe = xeA[:, g * 3 * NPG:(g + 1) * 3 * NPG]
            nc.vector.scalar_tensor_tensor(xe, m[:], -1.0, xg[:],
                                           op0=Alu.add, op1=Alu.max)
            # per-graph mean/var (DVE only; Ln/Exp batched to avoid activation
            # table swaps between exp and ln sets)
            xe3 = xe.rearrange("p (c n) -> p c n", c=3)
            s13 = wk.tile([128, 3], F32, tag="s13")
            nc.vector.tensor_reduce(s13[:], xe3, axis=mybir.AxisListType.X, op=Alu.add)
            sq = wk.tile([128, 3 * NPG], BF16, tag="sq")
            nc.vector.tensor_tensor(out=sq[:], in0=xe, in1=xe, op=Alu.mult)
            s23 = wk.tile([128, 3], F32, tag="s23")
            nc.vector.tensor_reduce(s23[:], sq[:].rearrange("p (c n) -> p c n", c=3),
                                    axis=mybir.AxisListType.X, op=Alu.add)
            mean3 = meanA[:, g * 3:(g + 1) * 3]
            nc.vector.tensor_scalar_mul(mean3, s13[:], 1.0 / NPG)
            msq3 = wk.tile([128, 3], F32, tag="msq3")
            nc.vector.tensor_tensor(out=msq3[:], in0=mean3, in1=mean3, op=Alu.mult)
            veps3 = vepsA[:, g * 3:(g + 1) * 3]
            nc.vector.scalar_tensor_tensor(veps3, s23[:], 1.0 / NPG, msq3[:],
                                           op0=Alu.mult, op1=Alu.subtract)
            nc.vector.tensor_scalar_add(veps3, veps3, 1e-5)

            def finish(gg):
                # out = xe * gisd - tcol   (gamma folded; beta==0)
                gisd3 = wk.tile([128, 3], F32, tag="gisd3")
                nc.vector.tensor_tensor(out=gisd3[:], in0=vepsA[:, gg * 3:(gg + 1) * 3],
                                        in1=gcol[:, 0:3], op=Alu.mult)
                tcol3 = wk.tile([128, 3], F32, tag="tcol3")
                nc.vector.tensor_tensor(out=tcol3[:], in0=meanA[:, gg * 3:(gg + 1) * 3],
                                        in1=gisd3[:], op=Alu.mult)
                for ck in range(3):
                    oc = xout[:, ck * NG + gg * NPG: ck * NG + (gg + 1) * NPG]
                    nc.vector.scalar_tensor_tensor(
                        oc, xeA[:, gg * 3 * NPG + ck * NPG: gg * 3 * NPG + (ck + 1) * NPG],
                        gisd3[:, ck:ck + 1],
                        tcol3[:, ck:ck + 1].broadcast_to((128, NPG)),
                        op0=Alu.mult, op1=Alu.subtract)

            if g == GPC - 2 or g == GPC - 1:
                # one Ln/Exp for graphs [0..4] at g==4, then [5] at g==5:
                # overwrite veps in place with 1/sqrt(veps)
                lo = 0 if g == GPC - 2 else (GPC - 1) * 3
                hi = (GPC - 1) * 3 if g == GPC - 2 else GPC * 3
                nc.scalar.activation(vepsA[:, lo:hi], vepsA[:, lo:hi], Act.Ln)
                nc.scalar.activation(vepsA[:, lo:hi], vepsA[:, lo:hi], Act.Exp,
                                     scale=-0.5)
                for gg in range(lo // 3, hi // 3):
                    finish(gg)
        return xout

    x2 = layer([xb[:, 0:NG], xb[:, NG:2 * NG]], w1s, was1, gncol, 0,
               hook_g1=emit_scalar_chunks)
    x2v = [x2[:, ck * NG:(ck + 1) * NG] for ck in range(3)]
    lin1_chunk(1, nc.sync)
    lin1_chunk(3, nc.sync)
    x3t = layer(x2v, w2s, was2, gncol2, 1)
    x3 = [x3t[:, ck * NG:(ck + 1) * NG] for ck in range(3)]

    # ---- lin1 GEMV: 684 k=128 chunks processed in PAIRS. Each pair loads a
    # full [128, 128] stationary tile (two adjacent nodes' weight chunks side
    # by side -> FWL-eligible) against a [128, 2, 6] moving slice. Diagonal
    # blocks of the [128, 12] psum hold the real partials; off-diagonal blocks
    # accumulate ignored cross terms. ----
    y_ps = psY.tile([128, 2 * GPC], F32, tag="y")
    for i in range(NLCH):
        lt = lin1_t[i]
        for jj in range(0, JPC, 2):
            jc = i * JPC + jj
            ck, n = jc // NPG, jc % NPG
            x3r = x3[ck].rearrange("p (g n) -> p n g", g=GPC)
            nc.tensor.matmul(y_ps[:], lt[:, jj * C:(jj + 2) * C],
                             x3r[:, n:n + 2, :],
                             start=(jc == 0), stop=(jc == NJ1 - 2))

    # fold: y = y_ps[0:64, 0:6] + y_ps[64:128, 6:12] (partition shift via DMA)
    yhi = wk.tile([128, GPC], F32, tag="yhi")
    nc.scalar.copy(yhi[64:128, :], y_ps[64:128, GPC:2 * GPC])
    ylo = wk.tile([C, GPC], F32, tag="ylo")
    nc.sync.dma_start(ylo[:], yhi[64:128, :])

    # ---- head: +b, elu, bn, lin2 ----
    yb = wk.tile([C, GPC], F32, tag="yb")
    nc.vector.scalar_tensor_tensor(yb[:], y_ps[0:C, 0:GPC], head64[:, 0:1],
                                   ylo[:], op0=Alu.add, op1=Alu.add)
    m2 = wk.tile([C, GPC], F32, tag="m2")
    nc.vector.tensor_scalar_min(m2[:], yb[:], 0.0)
    e2 = wk.tile([C, GPC], F32, tag="e2")
    nc.scalar.activation(e2[:], m2[:], Act.Exp)
    ye = wk.tile([C, GPC], F32, tag="ye")
    nc.vector.scalar_tensor_tensor(ye[:], e2[:], -1.0, yb[:], op0=Alu.add, op1=Alu.max)
    yn = wk.tile([C, GPC], F32, tag="yn")
    nc.vector.scalar_tensor_tensor(yn[:], ye[:], head64[:, 1:2],
                                   head64[:, 2:3].broadcast_to((C, GPC)),
                                   op0=Alu.mult, op1=Alu.add)
    o_ps = psY.tile([128, 2 * GPC], F32, tag="y")
    nc.tensor.matmul(o_ps[0:NCLS, 0:GPC], lin2w[:], yn[:], start=True, stop=True)
    ob = wk.tile([NCLS, GPC], F32, tag="ob")
    nc.vector.tensor_scalar_add(ob[:], o_ps[0:NCLS, 0:GPC], lin2b[:])
    nc.sync.dma_start(out_d.ap()[:, :], ob[:])

    for p in (psY, psT, psN, psZ, psS, psH, wk, xo, agw, scp, att, hp, lw, cst):
        p.release()


def _host_prep(inputs):
    """Build per-core input maps (sharding / relayout / dtype prep)."""
    import ml_dtypes
    x = np.asarray(inputs["x"], np.float32)
    ei = np.asarray(inputs["edge_index"])
    src, dst = np.asarray(ei[0], np.int64), np.asarray(ei[1], np.int64)

    # multiplicity matrices M[g, s, d] (+ self loops)
    g_of = src // NPG
    sl = src - g_of * NPG
    dl = dst - (dst // NPG) * NPG
    flat = g_of * (NPG * NPG) + sl * NPG + dl
    Mall = np.bincount(flat, minlength=B * NPG * NPG).astype(np.float32).reshape(B, NPG, NPG)
    Mall[:, np.arange(NPG), np.arange(NPG)] += 1.0

    xg = x.reshape(B, NPG, F_IN)

    def mk_asad(a_s, a_d):
        a_s = np.asarray(a_s, np.float32)
        a_d = np.asarray(a_d, np.float32)
        out = np.zeros((HC, 12), np.float32)
        for h in range(H):
            out[h * C:(h + 1) * C, h] = a_s[h]
            out[h * C:(h + 1) * C, 6 + h] = a_d[h]
        return out

    w1 = np.asarray(inputs["w1"], np.float32)
    w2 = np.asarray(inputs["w2"], np.float32)
    was1 = w1 @ mk_asad(inputs["as1"], inputs["ad1"])   # [228, 12]
    was2 = w2 @ mk_asad(inputs["as2"], inputs["ad2"])   # [384, 12]

    # kernel folds assume zero biases / unit mean-scale (true for this model)
    for nm in ("b1", "b2", "gn1_b", "gn2_b"):
        assert np.abs(np.asarray(inputs[nm])).max() == 0.0, f"{nm} nonzero"
    for nm in ("gn1_ms", "gn2_ms"):
        assert np.abs(np.asarray(inputs[nm]) - 1.0).max() == 0.0, f"{nm} != 1"

    bn_w = np.asarray(inputs["bn_w"], np.float64)
    bn_b = np.asarray(inputs["bn_b"], np.float64)
    bn_rm = np.asarray(inputs["bn_rm"], np.float64)
    bn_rv = np.asarray(inputs["bn_rv"], np.float64)
    bn_sc = bn_w / np.sqrt(bn_rv + 1e-5)
    bn_sh = bn_b - bn_rm * bn_sc
    head64 = np.stack([np.asarray(inputs["lin1_b"], np.float64),
                       bn_sc, bn_sh, np.zeros((C,))], axis=1).astype(np.float32)

    # lin1 reorder: rows j=(n*384 + ck*128 + p) -> chunks (ck, n) of k=128
    lwt = np.asarray(inputs["lin1_w"], np.float32).reshape(NPG, 3, 128, C)
    lin1s = np.ascontiguousarray(lwt.transpose(2, 1, 0, 3)).reshape(128, NJ1 * C) \
        .astype(ml_dtypes.bfloat16)

    def cm(a):
        """[g, n, f] -> [114 (f-part), (fc, g, n)] bf16 channel-major."""
        gg, nn, ff = a.shape
        nkc = ff // NH
        t = a.transpose(2, 0, 1).reshape(nkc, NH, gg, nn).transpose(1, 0, 2, 3)
        return np.ascontiguousarray(t).reshape(NH, nkc * gg * nn).astype(ml_dtypes.bfloat16)

    gnc1 = np.zeros((128, 4), np.float32)
    gnc2 = np.zeros((128, 4), np.float32)
    gnc1[:, 0:3] = np.asarray(inputs["gn1_w"], np.float32).reshape(3, 128).T
    gnc2[:, 0:3] = np.asarray(inputs["gn2_w"], np.float32).reshape(3, 128).T

    dpati = np.zeros((7, 6 * NG), np.float32)
    for j in range(7 - 1):
        dpati[j, j * NG:(j + 1) * NG] = 1.0
    shared = dict(
        dpati=dpati.astype(ml_dtypes.bfloat16),
        onesi=np.ones((1, NG), ml_dtypes.bfloat16),
        w1s=np.ascontiguousarray(
            w1.reshape(2, NH, HC).transpose(1, 0, 2)).reshape(NH, 2 * HC)
            .astype(ml_dtypes.bfloat16),
        w2s=np.ascontiguousarray(
            w2.reshape(3, 128, HC).transpose(1, 0, 2)).reshape(128, 3 * HC)
            .astype(ml_dtypes.bfloat16),
        was1=np.ascontiguousarray(
            was1.reshape(2, NH, 12).transpose(1, 0, 2)).reshape(NH, 24)
            .astype(ml_dtypes.bfloat16),
        was2=np.ascontiguousarray(
            was2.reshape(3, 128, 12).transpose(1, 0, 2)).reshape(128, 36)
            .astype(ml_dtypes.bfloat16),
        gncol=gnc1, gncol2=gnc2,
        lin1s=lin1s, head64=head64,
        lin2w=np.asarray(inputs["lin2_w"], np.float32),
        lin2b=np.asarray(inputs["lin2_b"], np.float32).reshape(NCLS, 1),
    )

    in_maps = []
    for core in range(NCORES):
        gs = slice(core * GPC, (core + 1) * GPC)
        m = dict(shared)
        m["xb"] = cm(xg[gs])                           # [114, (fc, g, n)]
        m["mm"] = cm(Mall[gs].transpose(0, 2, 1))      # [114 (s), (sc, g, d)]
        in_maps.append(m)
    return in_maps


_cached_nc = None


def kernel(**inputs):
    global _cached_nc
    in_maps = _host_prep(inputs)
    if _cached_nc is None:
        _cached_nc = _build_program()
    nc = _cached_nc
    res = bass_utils.run_bass_kernel_spmd(nc, in_maps, core_ids=list(range(NCORES)))
    _last_results["exec_time_ns"] = res.exec_time_ns
    _last_results["res"] = res
    out = np.zeros((B, NCLS), np.float32)
    for core in range(NCORES):
        o = res.results[core]["out"]          # [2, 6]
        out[core * GPC:(core + 1) * GPC, :] = o.T
    return out

